# revision 2
# baseline (speedup 1.0000x reference)
"""Trainium2 Bass kernel for nn_AudioImaginationForGLUE (v2, bf16).

Pure data-parallel across 8 NeuronCores: each core handles 4 samples
(B=32 / 8).  Spans are two sequential phases (span 1 may read
hidden-state rows written by span 0).

Math transformations (vs reference):
  - audio-MLP second layer folded into K/V projections:
       wk_eff = mlp_w2 @ wk,  wv_eff = mlp_w2 @ wv
  - key bias dropped (softmax shift invariance along key axis)
  - value bias folded into output-proj bias (softmax rows sum to 1)
  - attention scale folded into wq, bq
  - NO max subtraction in softmax (scores are small; verified on host)
  - scoresT computed directly in [token, query] layout:
       scoresT = h1 @ WQ',   WQ'_h = wk_eff_h @ q_h^T
    so no K tensor, no attention transposes, and the softmax
    normalization rides along the ctx matmul as a ones column in V.
  - ragged span handled by indirect-DMA gather/scatter with
    host-computed row indices.

Layout: activations transposed [feature -> partitions, tokens -> free].
Everything on the matmul path is bf16 (1 cycle/row on the PE, half the
DMA traffic, 2x DVE); PSUM accumulation is fp32.  Audio is loaded with
DMA-transpose (16-bit dtype) so the PE never transposes it.
"""

import numpy as np
import ml_dtypes

import concourse.bass as bass
import concourse.mybir as mybir
import concourse.tile as tile
from concourse import bacc
from concourse.masks import make_identity
from concourse.bass_utils import run_bass_kernel_spmd

F32 = mybir.dt.float32
BF16 = mybir.dt.bfloat16
I32 = mybir.dt.int32
AF = mybir.ActivationFunctionType
AX = mybir.AxisListType
OP = mybir.AluOpType

P = 128
B, S, H, NH, FF, A, TA, NSPAN, MAXL = 32, 512, 768, 12, 3072, 768, 1024, 2, 64
DH = H // NH          # 64
HC = H // P           # 6 hidden chunks
HP = NH // 2          # 6 head pairs
FC = FF // P          # 24 ffn chunks
TT = TA // P          # 8 audio token tiles
NCORES = 8
BPC = B // NCORES     # 4 samples per core
NB = BPC * MAXL       # 256, stage-B token width
SCALE = 1.0 / float(np.sqrt(DH))
NPBF = ml_dtypes.bfloat16


def build_program():
    nc = bacc.Bacc("TRN2", target_bir_lowering=False, debug=False)

    t = {}
    t["hs_in"] = nc.dram_tensor("hs_in", [BPC * S, H], F32, kind="ExternalInput")
    t["audio"] = nc.dram_tensor("audio", [BPC, NSPAN, TA, A], BF16,
                                kind="ExternalInput")
    for nm in ("w_mw1", "w_wv", "w_wq", "w_wo", "w_gaw", "w_gtw"):
        t[nm] = nc.dram_tensor(nm, [H, H], BF16, kind="ExternalInput")
    t["w_wkT"] = nc.dram_tensor("w_wkT", [P, HP * HC * P], BF16,
                                kind="ExternalInput")
    t["w_fw1"] = nc.dram_tensor("w_fw1", [H, FF], BF16, kind="ExternalInput")
    t["w_fw2"] = nc.dram_tensor("w_fw2", [FF, H], BF16, kind="ExternalInput")
    for nm in ("p_mb1", "p_bq", "p_fb2", "p_gb", "p_g1", "p_b1", "p_g2", "p_b2"):
        t[nm] = nc.dram_tensor(nm, [P, HC], F32, kind="ExternalInput")
    t["p_fb1"] = nc.dram_tensor("p_fb1", [P, FC], F32, kind="ExternalInput")
    t["bo_row"] = nc.dram_tensor("bo_row", [1, H], BF16, kind="ExternalInput")
    t["ones_c"] = nc.dram_tensor("ones_c", [P, 1], BF16, kind="ExternalInput")
    t["ones_r"] = nc.dram_tensor("ones_r", [1, NB], BF16, kind="ExternalInput")
    t["gidx"] = nc.dram_tensor("gidx", [NSPAN, BPC, MAXL], I32, kind="ExternalInput")
    t["vmsk"] = nc.dram_tensor("vmsk", [NSPAN, BPC, MAXL], F32, kind="ExternalInput")
    t["wmsk"] = nc.dram_tensor("wmsk", [NSPAN, BPC, MAXL], F32, kind="ExternalInput")
    t["hs_out"] = nc.dram_tensor("hs_out", [BPC * S, H], F32, kind="ExternalOutput")

    with tile.TileContext(nc) as tc, \
            nc.allow_low_precision("bf16 matmul path, fp32 accumulation"):
        _emit(nc, tc, t)
    nc.finalize()
    return nc


def _emit(nc, tc, t):
    hs_in, hs_out, audio = t["hs_in"], t["hs_out"], t["audio"]

    with (
        tc.tile_pool(name="const", bufs=1) as cpool,
        tc.tile_pool(name="resw", bufs=1) as resw,
        tc.tile_pool(name="perbs", bufs=1) as perbs,
        tc.tile_pool(name="pstg", bufs=1, space="PSUM") as pstg,
    ):
        # ---- constants ----
        identb = cpool.tile([P, P], BF16, tag="identb")
        make_identity(nc, identb)
        ones_col = cpool.tile([P, 1], BF16, tag="ones_col")
        nc.sync.dma_start(out=ones_col[:], in_=t["ones_c"][:, :])
        ones_row = cpool.tile([1, NB], BF16, tag="ones_row")
        nc.sync.dma_start(out=ones_row[:], in_=t["ones_r"][:, :])
        eps_t = cpool.tile([P, 1], F32, tag="eps_t")
        nc.vector.memset(eps_t[:], 1e-5)

        packs = {}
        for nm in ("p_mb1", "p_bq", "p_fb1", "p_fb2", "p_gb",
                   "p_g1", "p_b1", "p_g2", "p_b2"):
            nch = FC if nm == "p_fb1" else HC
            pk = cpool.tile([P, nch], F32, tag=nm)
            nc.sync.dma_start(out=pk[:], in_=t[nm][:, :])
            packs[nm] = pk
        borow = cpool.tile([1, H], BF16, tag="borow")
        nc.sync.dma_start(out=borow[:], in_=t["bo_row"][:, :])

        # ---- resident weights ----
        wres = {}
        for nm, dram in (("mw1", t["w_mw1"]), ("wv", t["w_wv"]),
                         ("wq", t["w_wq"])):
            ws = resw.tile([P, HC, H], BF16, tag="w_" + nm)
            nc.sync.dma_start(
                out=ws[:], in_=dram[:, :].rearrange("(c p) n -> p c n", p=P))
            wres[nm] = ws
        wkT = resw.tile([P, HP, HC, P], BF16, tag="w_wkT")
        nc.sync.dma_start(
            out=wkT[:], in_=t["w_wkT"][:, :].rearrange(
                "p (h c n) -> p h c n", h=HP, c=HC))
        wres["wkT"] = wkT

        # ---- full hidden-state copy in -> out (8 chunks) ----
        rows = BPC * S
        step = rows // 8
        for i in range(8):
            nc.sync.dma_start(out=hs_out[i * step:(i + 1) * step, :],
                              in_=hs_in[i * step:(i + 1) * step, :])

        st = _State(nc, t, cpool, perbs, pstg, resw, wres, packs, identb,
                    ones_col, ones_row, eps_t, borow)

        # prologue: audio + MLP1 for (span 0, sample 0)
        st.aiT_next = _audio_load(st, 0, 0)
        st.h1_next = _mlp1(st, st.aiT_next, pstg)

        for s in range(NSPAN):
            _emit_span(st, s)


class _State:
    def __init__(self, nc, t, cpool, perbs, pstg, resw, wres, packs, identb,
                 ones_col, ones_row, eps_t, borow):
        self.nc = nc
        self.t = t
        self.cpool = cpool
        self.perbs = perbs
        self.pstg = pstg
        self.resw = resw
        self.wres = wres
        self.packs = packs
        self.identb = identb
        self.ones_col = ones_col
        self.ones_row = ones_row
        self.eps_t = eps_t
        self.borow = borow
        self.aiT_next = None
        self.h1_next = None


def _audio_load(st, s, b):
    """DMA-transpose one sample-span of audio into [feat, tok] layout."""
    nc = st.nc
    aiT = st.perbs.tile([P, HC, TA], BF16, tag="aiT", bufs=2)
    for c in range(HC):
        nc.sync.dma_start(out=aiT[:, c, :],
                          in_=st.t["audio"][b, s, :, c * P:(c + 1) * P],
                          transpose=True)
    return aiT


def _mlp1(st, aiT, qpool):
    """h1T = relu(aiT.T @ mw1 + mb1) in transposed layout, bf16."""
    nc = st.nc
    h1T = st.perbs.tile([P, HC, TA], BF16, tag="h1T", bufs=2)
    for co in range(HC):
        for half in range(2):
            ph = qpool.tile([P, 512], F32, tag="stg", bufs=2)
            for ci in range(HC):
                nc.tensor.matmul(
                    ph[:, :], st.wres["mw1"][:, ci, co * P:(co + 1) * P],
                    aiT[:, ci, half * 512:(half + 1) * 512],
                    start=(ci == 0), stop=(ci == HC - 1))
            nc.vector.tensor_scalar(
                out=h1T[:, co, half * 512:(half + 1) * 512], in0=ph[:, :],
                scalar1=st.packs["p_mb1"][:, co:co + 1], scalar2=0.0,
                op0=OP.add, op1=OP.max)
    return h1T


def _emit_span(st, s):
    nc, t, perbs = st.nc, st.t, st.perbs

    spanT = perbs.tile([P, HC, BPC, MAXL], BF16, tag="spanT")
    ctxT = perbs.tile([P, HC, BPC, MAXL], BF16, tag="ctxT")
    gnat_t = [None] * BPC
    wm_t = [None] * BPC
    gi_t = [None] * BPC

    with (
        tc_pool(st, f"sA{s}") as pa,
        tc_pool(st, f"psA{s}", psum=True) as qa,
    ):
        qh = _phase_head(st, s, pa, qa, spanT, gnat_t, wm_t, gi_t)

        h1_t = [None] * BPC
        for b in range(BPC):
            if b == 0:
                aiT, h1T = st.aiT_next, st.h1_next
                st.aiT_next = st.h1_next = None
            else:
                aiT = _audio_load(st, s, b)
                h1T = _mlp1(st, aiT, qa)
            h1_t[b] = h1T
            _attend(st, s, b, pa, qa, qh, h1T, spanT, ctxT)

    with (
        tc_pool(st, f"sB{s}") as pb,
        tc_pool(st, f"psB{s}", psum=True) as qb,
    ):
        def stageb_cb(point, s=s):
            # overlap the next span's first audio load + MLP1 with the
            # fusion tail (PE fills stage-B's DVE/ACT latency holes)
            if s + 1 < NSPAN:
                if point == 0:
                    st.aiT_next = _audio_load(st, s + 1, 0)
                elif point == 1:
                    st.h1_next = _mlp1(st, st.aiT_next, st.pstg)

        _stage_b(st, s, pb, qb, spanT, ctxT, gnat_t, wm_t, gi_t, stageb_cb)


def tc_pool(st, name, psum=False):
    if psum:
        return st.nc.tc.tile_pool(name=name, bufs=1, space="PSUM")
    return st.nc.tc.tile_pool(name=name, bufs=1)


def _phase_head(st, s, pa, qa, spanT, gnat_t, wm_t, gi_t):
    """Gather the 4 span windows, build spanT, batched q projection."""
    nc, t = st.nc, st.t
    perbs = st.perbs
    src = t["hs_in"] if s == 0 else t["hs_out"]
    for b in range(BPC):
        gi = perbs.tile([MAXL, 1], I32, tag="gi", bufs=4)
        nc.sync.dma_start(out=gi[:],
                          in_=t["gidx"][s, b, :].rearrange("(p o) -> p o", o=1))
        vm = perbs.tile([MAXL, 1], F32, tag="vm", bufs=4)
        nc.sync.dma_start(out=vm[:],
                          in_=t["vmsk"][s, b, :].rearrange("(p o) -> p o", o=1))
        wm = perbs.tile([MAXL, 1], F32, tag="wm", bufs=4)
        nc.sync.dma_start(out=wm[:],
                          in_=t["wmsk"][s, b, :].rearrange("(p o) -> p o", o=1))
        gnat = perbs.tile([MAXL, H], F32, tag="gnat", bufs=4)
        nc.gpsimd.indirect_dma_start(
            out=gnat[:], out_offset=None, in_=src[:, :],
            in_offset=bass.IndirectOffsetOnAxis(ap=gi[:, :1], axis=0))
        gnat_t[b], wm_t[b], gi_t[b] = gnat, wm, gi

        snat = pa.tile([MAXL, H], BF16, tag="snat", bufs=2)
        nc.vector.tensor_scalar_mul(snat[:], gnat[:], vm[:, :1])
        for c in range(0, HC, 2):
            pt = qa.tile([P, 2, MAXL], BF16, tag="tp", bufs=2)
            for j in range(2):
                nc.tensor.transpose(out=pt[:, j, :],
                                    in_=snat[:, (c + j) * P:(c + j + 1) * P],
                                    identity=st.identb[:MAXL, :MAXL])
            nc.scalar.copy(spanT[:, c:c + 2, b, :], pt[:, :, :])

    # batched q projection -> qh [128(dh pair), head-pair, b, q], bf16
    qh = pa.tile([P, HP, BPC, MAXL], BF16, tag="qh", bufs=1)
    for co in range(HC):
        pq = qa.tile([P, 2, NB], F32, tag="mm", bufs=2)
        for ci in range(HC):
            nc.tensor.matmul(pq[:, 0, :], st.wres["wq"][:, ci, co * P:(co + 1) * P],
                             spanT[:, ci, :, :],
                             start=(ci == 0), stop=(ci == HC - 1))
        nc.scalar.activation(qh[:, co, :, :], pq[:, 0, :].rearrange(
            "p (b l) -> p b l", b=BPC), AF.Identity,
            bias=st.packs["p_bq"][:, co:co + 1])
    return qh


def _attend(st, s, b, pa, qa, qh, h1T, spanT, ctxT):
    """WQ' -> scoresT -> exp -> V -> ctx for one sample."""
    nc = st.nc
    wkT = st.wres["wkT"]

    # ---- WQ'[feat(c), (h, q)] = wk_eff_h @ q_h^T, bf16 ----
    wqp = pa.tile([P, HC, H], BF16, tag="wqp", bufs=2)
    for c in range(HC):
        pw = qa.tile([P, 2, 512], F32, tag="mm", bufs=2)
        for hp in range(HP):
            nc.tensor.matmul(
                pw[:, 0, :].rearrange("p (h n) -> p h n", h=4)[:, (2 * hp) % 4, :],
                wkT[0:DH, hp, c, :], qh[0:DH, hp, b, :],
                start=True, stop=True) if False else None
        # (loop unrolled explicitly below for clarity of psum offsets)
        for hp in range(HP):
            off = hp * 2 * MAXL
            nc.tensor.matmul(pw[:, 0, off:off + MAXL],
                             wkT[0:DH, hp, c, :], qh[0:DH, hp, b, :],
                             start=True, stop=True)
            nc.tensor.matmul(pw[:, 0, off + MAXL:off + 2 * MAXL],
                             wkT[DH:P, hp, c, :], qh[DH:P, hp, b, :],
                             start=True, stop=True)
        nc.scalar.copy(wqp[:, c, :], pw[:, 0, 0:H].rearrange(
            "p (c2 n) -> p c2 n", c2=2))
    # ---- scoresT tiles + exp -> esbT [tok, (h, q)], bf16 ----
    esbT = pa.tile([P, TT, H], BF16, tag="esbT", bufs=1)
    for tt in range(TT):
        ps = qa.tile([P, 2, 512], F32, tag="mm", bufs=2)
        for ci in range(HC):
            lhs = h1T[:, ci, tt * P:(tt + 1) * P]
            nc.tensor.matmul(ps[:, 0, :], lhs, wqp[:, ci, 0:512],
                             start=(ci == 0), stop=(ci == HC - 1))
            nc.tensor.matmul(ps[:, 1, 0:256], lhs, wqp[:, ci, 512:H],
                             start=(ci == 0), stop=(ci == HC - 1))
        nc.scalar.activation(esbT[:, tt, :], ps.ap_view(
            [(P, None), (512, 2), (1, 512)])[:, :, :] if False else
            ps[:, :, :].rearrange("p a n -> p (a n)")[:, 0:H].rearrange(
                "p (a n) -> p a n", a=2), AF.Exp) if False else None
        nc.scalar.activation(
            esbT[:, tt, 0:512], ps[:, 0, :], AF.Exp)
        nc.scalar.activation(
            esbT[:, tt, 512:H], ps[:, 1, 0:256], AF.Exp)

    # ---- V with ones column -> v_aug [tok, tt, h, 65], bf16 ----
    v_aug = pa.tile([P, TT, NH, DH + 1], BF16, tag="v_aug", bufs=2)
    nc.vector.memset(v_aug[:, :, :, DH:DH + 1], 1.0)
    for tt in range(TT):
        pv = qa.tile([P, 2, 512], F32, tag="mm", bufs=2)
        for ci in range(HC):
            lhs = h1T[:, ci, tt * P:(tt + 1) * P]
            nc.tensor.matmul(pv[:, 0, :], lhs, st.wres["wv"][:, ci, 0:512],
                             start=(ci == 0), stop=(ci == HC - 1))
            nc.tensor.matmul(pv[:, 1, 0:256], lhs, st.wres["wv"][:, ci, 512:H],
                             start=(ci == 0), stop=(ci == HC - 1))
        nc.vector.tensor_copy(
            v_aug[:, tt, 0:8, 0:DH], pv[:, 0, :].rearrange(
                "p (h d) -> p h d", h=8))
        nc.vector.tensor_copy(
            v_aug[:, tt, 8:NH, 0:DH], pv[:, 1, 0:256].rearrange(
                "p (h d) -> p h d", h=4))

    # ---- ctx + normalization -> ctxT[:, :, b, :] ----
    ctx_nat = pa.tile([MAXL, H], BF16, tag="ctx_nat", bufs=2)
    for hh in range(2):
        pc = qa.tile([MAXL, 512], F32, tag="ctxp", bufs=2)
        for h6 in range(6):
            h = hh * 6 + h6
            for tt in range(TT):
                nc.tensor.matmul(pc[:, h6 * (DH + 1):(h6 + 1) * (DH + 1)],
                                 esbT[:, tt, h * DH:(h + 1) * DH],
                                 v_aug[:, tt, h, :],
                                 start=(tt == 0), stop=(tt == TT - 1))
        for h6 in range(6):
            h = hh * 6 + h6
            rec = pa.tile([MAXL, 1], F32, tag="rec", bufs=4)
            nc.vector.reciprocal(rec[:], pc[:, h6 * (DH + 1) + DH:
                                            h6 * (DH + 1) + DH + 1])
            nc.vector.tensor_scalar_mul(
                ctx_nat[:, h * DH:(h + 1) * DH],
                pc[:, h6 * (DH + 1):h6 * (DH + 1) + DH], rec[:, :1])
    for c in range(0, HC, 2):
        pt = qa.tile([P, 2, MAXL], BF16, tag="tp", bufs=2)
        for j in range(2):
            nc.tensor.transpose(out=pt[:, j, :],
                                in_=ctx_nat[:, (c + j) * P:(c + j + 1) * P],
                                identity=st.identb[:MAXL, :MAXL])
        nc.scalar.copy(ctxT[:, c:c + 2, b, :], pt[:, :, :])


def _layernorm_T(st, qb, pb, xT, outT, gpack, bpack):
    """LayerNorm over the partition (feature) axis of xT [128, HC, BPC*MAXL]."""
    nc = st.nc
    psum = qb.tile([1, NB], F32, tag="st", bufs=2)
    for c in range(HC):
        nc.tensor.matmul(psum[:, :], st.ones_col[:, :], xT[:, c, :, :],
                         start=(c == 0), stop=(c == HC - 1))
    m_row = pb.tile([1, NB], BF16, tag="m_row", bufs=1)
    nc.vector.tensor_scalar_mul(m_row[:], psum[:, :], 1.0 / H)

    sq = pb.tile([P, HC, NB], BF16, tag="sq", bufs=1)
    for c in range(HC):
        nc.scalar.activation(sq[:, c, :], xT[:, c, :, :], AF.Square)
    psq = qb.tile([1, NB], F32, tag="st", bufs=2)
    for c in range(HC):
        nc.tensor.matmul(psq[:, :], st.ones_col[:, :], sq[:, c, :],
                         start=(c == 0), stop=(c == HC - 1))
    var = pb.tile([1, NB], BF16, tag="var", bufs=1)
    msq = pb.tile([1, NB], F32, tag="msq", bufs=1)
    nc.scalar.activation(msq[:], m_row[:], AF.Square)
    nc.vector.tensor_scalar(out=var[:], in0=psq[:, :], scalar1=1.0 / H,
                            scalar2=None, op0=OP.mult)
    nc.vector.tensor_tensor(out=var[:], in0=var[:], in1=msq[:],
                            op=OP.subtract)
    # broadcast mean / variance to all partitions via rank-1 matmuls
    pm_b = qb.tile([P, NB], F32, tag="st2", bufs=2)
    nc.tensor.matmul(pm_b[:, :], st.ones_row[:1, :P], m_row[:1, :],
                     start=True, stop=True)
    pv_b = qb.tile([P, NB], F32, tag="st2", bufs=2)
    nc.tensor.matmul(pv_b[:, :], st.ones_row[:1, :P], var[:1, :],
                     start=True, stop=True)
    rstd = pb.tile([P, NB], F32, tag="rstd", bufs=1)
    nc.scalar.activation(rstd[:], pv_b[:, :], AF.Sqrt, bias=st.eps_t[:, :1])
    nc.vector.reciprocal(rstd[:], rstd[:])
    for c in range(HC):
        nc.vector.tensor_tensor(out=outT[:, c, :, :], in0=xT[:, c, :, :],
                                in1=pm_b[:, :].rearrange(
                                    "p (b l) -> p b l", b=BPC),
                                op=OP.subtract)
        nc.vector.tensor_tensor(out=outT[:, c, :, :], in0=outT[:, c, :, :],
                                in1=rstd[:, :].rearrange(
                                    "p (b l) -> p b l", b=BPC),
                                op=OP.mult)
        nc.vector.tensor_scalar(out=outT[:, c, :, :], in0=outT[:, c, :, :],
                                scalar1=gpack[:, c:c + 1],
                                scalar2=bpack[:, c:c + 1],
                                op0=OP.mult, op1=OP.add)


def _stage_b(st, s, pb, qb, spanT, ctxT, gnat_t, wm_t, gi_t, stageb_cb):
    """Batched (over b) fusion tail: o-proj, LN1, FFN, LN2, gates, merge."""
    nc, t = st.nc, st.t
    packs = st.packs
    hs_out = t["hs_out"]

    # ---- o = ctx @ wo + bo  (+ residual span) -> x1 ----
    x1 = pb.tile([P, HC, BPC, MAXL], BF16, tag="xT", bufs=2)
    for co in range(HC):
        wc = pb.tile([P, HC, P], BF16, tag="wcol", bufs=3)
        nc.sync.dma_start(
            out=wc[:], in_=t["w_wo"][:, co * P:(co + 1) * P]
            .rearrange("(c p) n -> p c n", p=P))
        po = qb.tile([P, NB], F32, tag="mmB", bufs=3)
        for ci in range(HC):
            nc.tensor.matmul(po[:, :], wc[:, ci, :],
                             ctxT[:, ci, :, :],
                             start=(ci == 0), stop=False)
        nc.tensor.matmul(po[:, :], st.borow[:1, co * P:(co + 1) * P],
                         st.ones_row[:1, :], start=False, stop=True)
        nc.vector.tensor_tensor(out=x1[:, co, :, :],
                                in0=po[:, :].rearrange(
                                    "p (b l) -> p b l", b=BPC),
                                in1=spanT[:, co, :, :], op=OP.add)

    stageb_cb(0)

    # ---- LN1 ----
    o1 = pb.tile([P, HC, BPC, MAXL], BF16, tag="out1T", bufs=1)
    _layernorm_T(st, qb, pb, x1, o1, packs["p_g1"], packs["p_b1"])

    # ---- FFN: per cf: h = gelu(o1 @ fw1_cf), acc += h @ fw2_cf ----
    GRP = 8
    acc = pb.tile([P, HC, NB], F32, tag="acc", bufs=1)
    for sup in range(FC // GRP):
        for j in range(GRP):
            cf = sup * GRP + j
            f1 = pb.tile([P, HC, P], BF16, tag="f1c", bufs=3)
            nc.sync.dma_start(
                out=f1[:], in_=t["w_fw1"][:, cf * P:(cf + 1) * P]
                .rearrange("(c p) n -> p c n", p=P))
            ph = qb.tile([P, NB], F32, tag="mmB", bufs=3)
            for ci in range(HC):
                nc.tensor.matmul(ph[:, :], f1[:, ci, :], o1[:, ci, :, :],
                                 start=(ci == 0), stop=(ci == HC - 1))
            hf = pb.tile([P, NB], BF16, tag="hf", bufs=3)
            nc.scalar.activation(hf[:, :], ph[:, :], AF.Gelu,
                                 bias=packs["p_fb1"][:, cf:cf + 1])
            f2c = pb.tile([P, H], BF16, tag="f2c", bufs=3)
            nc.scalar.dma_start(out=f2c[:], in_=t["w_fw2"][cf * P:(cf + 1) * P, :])
            for co in range(HC):
                pacc = qb.tile([P, 2, NB], F32, tag="acc2", bufs=3,
                               no_rotate=(j > 0 or co > 0))
                nc.tensor.matmul(pacc[:, co % 2, :] if False else
                                 pacc[:, 0, :] if False else
                                 pacc.ap()[:, :, :][:, co % 2, :],
                                 f2c[:, co * P:(co + 1) * P], hf[:, :],
                                 start=(j == 0), stop=(j == GRP - 1))
        if True:
            pass
    # (rewritten cleanly below -- see _stage_b_ffn)


def _noop():
    pass


# revision 35
# speedup vs baseline: 1.0300x; 1.0300x over previous
"""Trainium2 Bass kernel for nn_AudioImaginationForGLUE (v2, bf16).

Pure data-parallel across 8 NeuronCores: each core handles 4 samples
(B=32 / 8).  Spans are two sequential phases (span 1 may read
hidden-state rows written by span 0).

Math transformations (vs reference):
  - audio-MLP second layer folded into K/V projections:
       wk_eff = mlp_w2 @ wk,  wv_eff = mlp_w2 @ wv
  - key bias dropped (softmax shift invariance along key axis)
  - value bias folded into output-proj bias (softmax rows sum to 1)
  - attention scale folded into wq, bq
  - NO max subtraction in softmax (scores are small; verified on host)
  - scoresT computed directly in [token, query] layout:
       scoresT = h1 @ WQ',   WQ'_h = wk_eff_h @ q_h^T
    so no K tensor, no attention transposes; the softmax normalization
    rides along the ctx matmul as a ones column appended to V.
  - ragged span handled by indirect-DMA gather/scatter with
    host-computed row indices.

Layout: activations transposed [feature -> partitions, tokens -> free].
Everything on the matmul path is bf16 (1 cycle/row on the PE, half the
DMA traffic, 2x DVE); accumulation is fp32 in PSUM.  Audio is loaded
with DMA-transpose (16-bit dtype) so the PE never transposes it.
"""

import numpy as np
import ml_dtypes

import concourse.bass as bass
import concourse.mybir as mybir
import concourse.tile as tile
from concourse import bacc
from concourse.masks import make_identity
from concourse.bass_utils import run_bass_kernel_spmd

F32 = mybir.dt.float32
BF16 = mybir.dt.bfloat16
I32 = mybir.dt.int32
AF = mybir.ActivationFunctionType
AX = mybir.AxisListType
OP = mybir.AluOpType

P = 128
B, S, H, NH, FF, A, TA, NSPAN, MAXL = 32, 512, 768, 12, 3072, 768, 1024, 2, 64
DH = H // NH          # 64
HC = H // P           # 6 hidden chunks
HP = NH // 2          # 6 head pairs
FC = FF // P          # 24 ffn chunks
TT = TA // P          # 8 audio token tiles
NCORES = 8
BPC = B // NCORES     # 4 samples per core
NB = BPC * MAXL       # 256, stage-B token width
SCALE = 1.0 / float(np.sqrt(DH))
NPBF = ml_dtypes.bfloat16


def build_program():
    nc = bacc.Bacc("TRN2", target_bir_lowering=False, debug=False)

    t = {}
    t["hs_in"] = nc.dram_tensor("hs_in", [BPC * S, H], F32, kind="ExternalInput")
    t["audio"] = nc.dram_tensor("audio", [BPC, NSPAN, TA, A], BF16,
                                kind="ExternalInput")
    for nm in ("w_mw1", "w_wv", "w_wq", "w_wo", "w_gaw", "w_gtw"):
        t[nm] = nc.dram_tensor(nm, [H, H], BF16, kind="ExternalInput")
    t["w_wkT"] = nc.dram_tensor("w_wkT", [P, HP * HC * P], BF16,
                                kind="ExternalInput")
    t["w_fw1"] = nc.dram_tensor("w_fw1", [H, FF], BF16, kind="ExternalInput")
    t["w_fw2"] = nc.dram_tensor("w_fw2", [FF, H], BF16, kind="ExternalInput")
    for nm in ("p_mb1", "p_bq", "p_fb2", "p_gb", "p_g1", "p_b1", "p_g2", "p_b2"):
        t[nm] = nc.dram_tensor(nm, [P, HC], F32, kind="ExternalInput")
    t["p_fb1"] = nc.dram_tensor("p_fb1", [P, FC], F32, kind="ExternalInput")
    t["bo_row"] = nc.dram_tensor("bo_row", [1, H], BF16, kind="ExternalInput")
    t["ones_c"] = nc.dram_tensor("ones_c", [P, 1], BF16, kind="ExternalInput")
    t["ones_r"] = nc.dram_tensor("ones_r", [1, NB], BF16, kind="ExternalInput")
    t["gidx"] = nc.dram_tensor("gidx", [NSPAN, BPC, MAXL], I32, kind="ExternalInput")
    t["vmsk"] = nc.dram_tensor("vmsk", [NSPAN, BPC, MAXL], F32, kind="ExternalInput")
    t["wmsk"] = nc.dram_tensor("wmsk", [NSPAN, BPC, MAXL], F32, kind="ExternalInput")
    t["hs_out"] = nc.dram_tensor("hs_out", [BPC * S, H], F32, kind="ExternalOutput")
    if DEBUG:
        t["d_h1"] = nc.dram_tensor("d_h1", [P, HC, TA], BF16,
                                   kind="ExternalOutput")
        t["d_wqp"] = nc.dram_tensor("d_wqp", [P, HC, H], BF16,
                                    kind="ExternalOutput")
        t["d_esb"] = nc.dram_tensor("d_esb", [P, TT, H], BF16,
                                    kind="ExternalOutput")
        t["d_vaug"] = nc.dram_tensor("d_vaug", [P, TT, NH, DH + 1], BF16,
                                     kind="ExternalOutput")
        t["d_ctx"] = nc.dram_tensor("d_ctx", [MAXL, H], BF16,
                                    kind="ExternalOutput")
        t["d_span"] = nc.dram_tensor("d_span", [P, HC, BPC, MAXL], BF16,
                                     kind="ExternalOutput")
        t["d_qbd"] = nc.dram_tensor("d_qbd", [P, HP, BPC, 2, MAXL], BF16,
                                    kind="ExternalOutput")
        t["d_x1"] = nc.dram_tensor("d_x1", [P, HC, BPC, MAXL], BF16,
                                   kind="ExternalOutput")
        t["d_o1"] = nc.dram_tensor("d_o1", [P, HC, BPC, MAXL], BF16,
                                   kind="ExternalOutput")
        t["d_o2"] = nc.dram_tensor("d_o2", [P, HC, BPC, MAXL], BF16,
                                   kind="ExternalOutput")
        t["d_gate"] = nc.dram_tensor("d_gate", [P, HC, BPC, MAXL], BF16,
                                     kind="ExternalOutput")
        t["d_x2"] = nc.dram_tensor("d_x2", [P, HC, BPC, MAXL], BF16,
                                   kind="ExternalOutput")

    with tile.TileContext(nc) as tc, \
            nc.allow_low_precision("bf16 matmul path, fp32 accumulation"):
        _emit(nc, tc, t)
    nc.finalize()
    return nc


class _St:
    pass


def _emit(nc, tc, t):
    hs_in, hs_out = t["hs_in"], t["hs_out"]

    with (
        tc.tile_pool(name="const", bufs=1) as cpool,
        tc.tile_pool(name="resw", bufs=1) as resw,
        tc.tile_pool(name="perbs", bufs=1) as perbs,
    ):
        st = _St()
        st.nc, st.tc, st.t, st.perbs = nc, tc, t, perbs

        # ---- constants ----
        st.identb = cpool.tile([P, P], BF16, tag="identb")
        make_identity(nc, st.identb)
        st.ones_col = cpool.tile([P, 1], BF16, tag="ones_col")
        nc.sync.dma_start(out=st.ones_col[:], in_=t["ones_c"][:, :])
        st.ones_row = cpool.tile([1, NB], BF16, tag="ones_row")
        nc.sync.dma_start(out=st.ones_row[:], in_=t["ones_r"][:, :])
        st.eps_t = cpool.tile([P, 1], F32, tag="eps_t")
        nc.vector.memset(st.eps_t[:], 1e-5)

        st.packs = {}
        for nm in ("p_mb1", "p_bq", "p_fb1", "p_fb2", "p_gb",
                   "p_g1", "p_b1", "p_g2", "p_b2"):
            nch = FC if nm == "p_fb1" else HC
            pk = cpool.tile([P, nch], F32, tag=nm)
            nc.sync.dma_start(out=pk[:], in_=t[nm][:, :])
            st.packs[nm] = pk
        st.borow = cpool.tile([1, H], BF16, tag="borow")
        nc.sync.dma_start(out=st.borow[:], in_=t["bo_row"][:, :])

        # ---- resident weights ----
        st.wres = {}
        for nm, dram in (("mw1", t["w_mw1"]), ("wv", t["w_wv"]),
                         ("wq", t["w_wq"])):
            ws = resw.tile([P, HC, H], BF16, tag="w_" + nm)
            nc.sync.dma_start(
                out=ws[:], in_=dram[:, :].rearrange("(c p) n -> p c n", p=P))
            st.wres[nm] = ws
        wkT = resw.tile([P, HP, HC, P], BF16, tag="w_wkT")
        nc.sync.dma_start(
            out=wkT[:], in_=t["w_wkT"][:, :].rearrange(
                "p (h c n) -> p h c n", h=HP, c=HC))
        st.wres["wkT"] = wkT

        # ---- full hidden-state copy in -> out (8 chunks) ----
        rows = BPC * S
        step = rows // 8
        for i in range(8):
            nc.sync.dma_start(out=hs_out[i * step:(i + 1) * step, :],
                              in_=hs_in[i * step:(i + 1) * step, :])

        # ---- prologue: audio + MLP1 for (span 0, sample 0) ----
        with tc.tile_pool(name="pstg", bufs=1, space="PSUM") as pstg:
            st.aiT_next = _audio_load(st, 0, 0, pstg)
            st.h1_next = _mlp1(st, st.aiT_next, pstg)

        for s in range(NSPAN):
            _emit_span(st, s)


USE_DMA_T = False
DEBUG = False


def _audio_load(st, s, b, qpool):
    """Load one sample-span of audio into [feat, tok] layout."""
    nc = st.nc
    aiT = st.perbs.tile([P, HC, TA], BF16, tag="aiT", bufs=2)
    if USE_DMA_T:
        for c in range(HC):
            nc.sync.dma_start(out=aiT[:, c, :],
                              in_=st.t["audio"][b, s, :, c * P:(c + 1) * P],
                              transpose=True)
        return aiT
    for tt in range(TT):
        anat = st.perbs.tile([P, A], BF16, tag="anat", bufs=3)
        nc.sync.dma_start(out=anat[:], in_=st.t["audio"][b, s,
                                                         tt * P:(tt + 1) * P, :])
        for c in range(0, HC, 2):
            pt = qpool.tile([P, 2, P], BF16, tag="tp", bufs=2)
            for j in range(2):
                nc.tensor.transpose(out=pt[:, j, :],
                                    in_=anat[:, (c + j) * P:(c + j + 1) * P],
                                    identity=st.identb[:, :])
            nc.scalar.copy(aiT[:, c:c + 2, tt * P:(tt + 1) * P],
                           pt[:, :, :])
    return aiT


def _mlp1(st, aiT, qpool):
    """h1T = relu(aiT.T @ mw1 + mb1) in transposed layout, bf16."""
    nc = st.nc
    h1T = st.perbs.tile([P, HC, TA], BF16, tag="h1T", bufs=2)
    for co in range(HC):
        for half in range(2):
            ph = qpool.tile([P, 1024], F32, tag="mm", bufs=2)
            for ci in range(HC):
                nc.tensor.matmul(
                    ph[:, 0:512], st.wres["mw1"][:, ci, co * P:(co + 1) * P],
                    aiT[:, ci, half * 512:(half + 1) * 512],
                    start=(ci == 0), stop=(ci == HC - 1))
            nc.vector.tensor_scalar(
                out=h1T[:, co, half * 512:(half + 1) * 512], in0=ph[:, 0:512],
                scalar1=st.packs["p_mb1"][:, co:co + 1], scalar2=0.0,
                op0=OP.add, op1=OP.max)
    return h1T


def _emit_span(st, s):
    nc, perbs = st.nc, st.perbs

    spanT = perbs.tile([P, HC, BPC, MAXL], BF16, tag="spanT")
    ctxT = perbs.tile([P, HC, BPC, MAXL], BF16, tag="ctxT")
    gnat_t = [None] * BPC
    wm_t = [None] * BPC
    gi_t = [None] * BPC

    with (
        st.tc.tile_pool(name=f"sA{s}", bufs=1) as pa,
        st.tc.tile_pool(name=f"psA{s}", bufs=1, space="PSUM") as qa,
    ):
        qh = _phase_head(st, s, pa, qa, spanT, gnat_t, wm_t, gi_t)
        for b in range(BPC):
            if st.h1_next is not None:
                aiT, h1T = st.aiT_next, st.h1_next
                st.aiT_next = st.h1_next = None
            else:
                aiT = st.aiT_next if st.aiT_next is not None else \
                    _audio_load(st, s, b, qa)
                st.aiT_next = None
                h1T = _mlp1(st, aiT, qa)
            if b + 1 < BPC:
                st.aiT_next = _audio_load(st, s, b + 1, qa)
            _attend(st, s, b, pa, qa, qh, h1T, spanT, ctxT)

    with (
        st.tc.tile_pool(name=f"sB{s}", bufs=1) as pb,
        st.tc.tile_pool(name=f"psB{s}", bufs=1, space="PSUM") as qb,
    ):
        def stageb_cb(point, s=s):
            pass

        _stage_b(st, s, pb, qb, spanT, ctxT, gnat_t, wm_t, gi_t, stageb_cb)


def _phase_head(st, s, pa, qa, spanT, gnat_t, wm_t, gi_t):
    """Gather the 4 span windows, build spanT, batched q projection."""
    nc, t = st.nc, st.t
    perbs = st.perbs
    src = t["hs_in"] if s == 0 else t["hs_out"]
    for b in range(BPC):
        gi = perbs.tile([MAXL, 1], I32, tag="gi", bufs=4)
        nc.sync.dma_start(out=gi[:],
                          in_=t["gidx"][s, b, :].rearrange("(p o) -> p o", o=1))
        vm = perbs.tile([MAXL, 1], F32, tag="vm", bufs=4)
        nc.sync.dma_start(out=vm[:],
                          in_=t["vmsk"][s, b, :].rearrange("(p o) -> p o", o=1))
        wm = perbs.tile([MAXL, 1], F32, tag="wm", bufs=4)
        nc.sync.dma_start(out=wm[:],
                          in_=t["wmsk"][s, b, :].rearrange("(p o) -> p o", o=1))
        gnat = perbs.tile([MAXL, H], F32, tag="gnat", bufs=4)
        nc.gpsimd.indirect_dma_start(
            out=gnat[:], out_offset=None, in_=src[:, :],
            in_offset=bass.IndirectOffsetOnAxis(ap=gi[:, :1], axis=0))
        gnat_t[b], wm_t[b], gi_t[b] = gnat, wm, gi

        snat = pa.tile([MAXL, H], BF16, tag="snat", bufs=2)
        nc.vector.tensor_scalar_mul(snat[:], gnat[:], vm[:, :1])
        for c in range(0, HC, 2):
            pt = qa.tile([P, 2, P], BF16, tag="tp", bufs=2)
            for j in range(2):
                nc.tensor.transpose(out=pt[:, j, 0:MAXL],
                                    in_=snat[:, (c + j) * P:(c + j + 1) * P],
                                    identity=st.identb[:MAXL, :MAXL])
            nc.scalar.copy(spanT[:, c:c + 2, b, :], pt[:, :, 0:MAXL])

    # batched q projection into block-diagonal layout: for each head pair
    # the [128, 2, MAXL] slice [:, hp, b, :, :] is [[qA, 0], [0, qB]], so a
    # single K=128 matmul against the wkT pair computes both heads' WQ'
    # columns without mixing them.
    qbd = pa.tile([P, HP, BPC, 2, MAXL], BF16, tag="qbd", bufs=1)
    nc.vector.memset(qbd[0:DH, :, :, 1, :], 0.0)
    nc.vector.memset(qbd[DH:P, :, :, 0, :], 0.0)
    for co in range(HC):
        pq = qa.tile([P, 1024], F32, tag="mm", bufs=2)
        for ci in range(HC):
            nc.tensor.matmul(pq[:, 0:NB],
                             st.wres["wq"][:, ci, co * P:(co + 1) * P],
                             spanT[:, ci, :, :],
                             start=(ci == 0), stop=(ci == HC - 1))
        nc.scalar.activation(qbd[0:DH, co, :, 0, :], pq[0:DH, 0:NB],
                             AF.Identity, bias=st.packs["p_bq"][0:DH, co:co + 1])
        nc.scalar.activation(qbd[DH:P, co, :, 1, :], pq[DH:P, 0:NB],
                             AF.Identity, bias=st.packs["p_bq"][DH:P, co:co + 1])
    if DEBUG and s == 0:
        nc.sync.dma_start(out=t["d_span"][:, :, :, :], in_=spanT[:, :, :, :])
        nc.sync.dma_start(out=t["d_qbd"][:, :, :, :, :],
                          in_=qbd[:, :, :, :, :])
    return qbd


def _attend(st, s, b, pa, qa, qh, h1T, spanT, ctxT):
    """WQ' -> scoresT -> exp -> V -> ctx for one sample."""
    nc = st.nc
    wkT = st.wres["wkT"]

    # ---- WQ'[feat(c chunk), (head, q)] = wk_eff_h @ q_h^T, bf16 ----
    wqp = pa.tile([P, HC, H], BF16, tag="wqp", bufs=1)
    for c in range(HC):
        pw = qa.tile([P, 1024], F32, tag="mm", bufs=2)
        for hp in range(HP):
            nc.tensor.matmul(pw[:, hp * P:(hp + 1) * P],
                             wkT[:, hp, c, :], qh[:, hp, b, :, :],
                             start=True, stop=True)
        nc.scalar.copy(wqp[:, c, :], pw[:, 0:H])

    # ---- scoresT tiles + exp -> esbT [tok, (head, q)], bf16 ----
    esbT = pa.tile([P, TT, H], BF16, tag="esbT", bufs=1)
    for tt in range(TT):
        ps = qa.tile([P, 1024], F32, tag="mm", bufs=2)
        for ci in range(HC):
            lhs = h1T[:, ci, tt * P:(tt + 1) * P]
            nc.tensor.matmul(ps[:, 0:512], lhs, wqp[:, ci, 0:512],
                             start=(ci == 0), stop=(ci == HC - 1))
            nc.tensor.matmul(ps[:, 512:H], lhs, wqp[:, ci, 512:H],
                             start=(ci == 0), stop=(ci == HC - 1))
        nc.scalar.activation(esbT[:, tt, :], ps[:, 0:H], AF.Exp)

    # ---- V with ones column -> v_aug [tok, tt, head, 65], bf16 ----
    v_aug = pa.tile([P, TT, NH, DH + 1], BF16, tag="v_aug", bufs=1)
    nc.vector.memset(v_aug[:, :, :, DH:DH + 1], 1.0)
    for tt in range(TT):
        pv = qa.tile([P, 1024], F32, tag="mm", bufs=2)
        for ci in range(HC):
            lhs = h1T[:, ci, tt * P:(tt + 1) * P]
            nc.tensor.matmul(pv[:, 0:512], lhs, st.wres["wv"][:, ci, 0:512],
                             start=(ci == 0), stop=(ci == HC - 1))
            nc.tensor.matmul(pv[:, 512:H], lhs, st.wres["wv"][:, ci, 512:H],
                             start=(ci == 0), stop=(ci == HC - 1))
        nc.vector.tensor_copy(v_aug[:, tt, 0:8, 0:DH],
                              pv[:, 0:512].rearrange("p (h d) -> p h d", h=8))
        nc.vector.tensor_copy(v_aug[:, tt, 8:NH, 0:DH],
                              pv[:, 512:H].rearrange("p (h d) -> p h d", h=4))

    # ---- ctx + normalization -> ctxT[:, :, b, :] ----
    ctx_nat = pa.tile([MAXL, H], BF16, tag="ctx_nat", bufs=2)
    for hh in range(2):
        pc = qa.tile([MAXL, 512], F32, tag="ctxp", bufs=2)
        for h6 in range(6):
            h = hh * 6 + h6
            for tt in range(TT):
                nc.tensor.matmul(pc[:, h6 * (DH + 1):(h6 + 1) * (DH + 1)],
                                 esbT[:, tt, h * DH:(h + 1) * DH],
                                 v_aug[:, tt, h, :],
                                 start=(tt == 0), stop=(tt == TT - 1))
        for h6 in range(6):
            h = hh * 6 + h6
            rec = pa.tile([MAXL, 1], F32, tag="rec", bufs=4)
            nc.vector.reciprocal(
                rec[:], pc[:, h6 * (DH + 1) + DH:h6 * (DH + 1) + DH + 1])
            nc.vector.tensor_scalar_mul(
                ctx_nat[:, h * DH:(h + 1) * DH],
                pc[:, h6 * (DH + 1):h6 * (DH + 1) + DH], rec[:, :1])
    for c in range(0, HC, 2):
        pt = qa.tile([P, 2, P], BF16, tag="tp", bufs=2)
        for j in range(2):
            nc.tensor.transpose(out=pt[:, j, 0:MAXL],
                                in_=ctx_nat[:, (c + j) * P:(c + j + 1) * P],
                                identity=st.identb[:MAXL, :MAXL])
        nc.scalar.copy(ctxT[:, c:c + 2, b, :], pt[:, :, 0:MAXL])

    if DEBUG and s == 0 and b == 0:
        nc.sync.dma_start(out=st.t["d_h1"][:, :, :], in_=h1T[:, :, :])
        nc.sync.dma_start(out=st.t["d_wqp"][:, :, :], in_=wqp[:, :, :])
        nc.sync.dma_start(out=st.t["d_esb"][:, :, :], in_=esbT[:, :, :])
        nc.sync.dma_start(out=st.t["d_vaug"][:, :, :, :], in_=v_aug[:, :, :, :])
        nc.sync.dma_start(out=st.t["d_ctx"][:, :], in_=ctx_nat[:, :])


def _layernorm_T(st, qb, pb, xT, outT, gpack, bpack):
    """LayerNorm over the partition (feature) axis of xT [128, HC, BPC, MAXL].

    xT is bf16; column stats via ones-matmul, partition-broadcast of the
    normalization rows via rank-1 matmuls.
    """
    nc = st.nc
    pst = qb.tile([1, 2, NB], F32, tag="st", bufs=1)
    for c in range(HC):
        nc.tensor.matmul(pst[:, 0, :], st.ones_col[:, :], xT[:, c, :, :],
                         start=(c == 0), stop=(c == HC - 1))
    m_row = pb.tile([1, NB], BF16, tag="m_row", bufs=1)
    nc.vector.tensor_scalar_mul(m_row[:], pst[:, 0, :], 1.0 / H)

    sq = pb.tile([P, HC, NB], BF16, tag="sq", bufs=1)
    for c in range(HC):
        nc.scalar.activation(sq[:, c, :], xT[:, c, :, :], AF.Square)
    for c in range(HC):
        nc.tensor.matmul(pst[:, 1, :], st.ones_col[:, :], sq[:, c, :],
                         start=(c == 0), stop=(c == HC - 1))
    var = pb.tile([1, NB], BF16, tag="var", bufs=1)
    msq = pb.tile([1, NB], F32, tag="msq", bufs=1)
    nc.scalar.activation(msq[:], m_row[:], AF.Square)
    nc.vector.tensor_scalar(out=var[:], in0=pst[:, 1, :], scalar1=1.0 / H,
                            scalar2=None, op0=OP.mult)
    nc.vector.tensor_tensor(out=var[:], in0=var[:], in1=msq[:],
                            op=OP.subtract)
    # broadcast mean and variance to all partitions via rank-1 matmuls
    pmv = qb.tile([P, 2, NB], F32, tag="st2", bufs=1)
    nc.tensor.matmul(pmv[:, 0, :], st.ones_row[:1, :P], m_row[:1, :],
                     start=True, stop=True)
    nc.tensor.matmul(pmv[:, 1, :], st.ones_row[:1, :P], var[:1, :],
                     start=True, stop=True)
    rstd = pb.tile([P, NB], F32, tag="rstd", bufs=1)
    nc.scalar.activation(rstd[:], pmv[:, 1, :], AF.Sqrt, bias=st.eps_t[:, :1])
    nc.vector.reciprocal(rstd[:], rstd[:])
    for c in range(HC):
        nc.vector.tensor_tensor(out=outT[:, c, :, :], in0=xT[:, c, :, :],
                                in1=pmv[:, 0, :], op=OP.subtract)
        nc.vector.tensor_tensor(out=outT[:, c, :, :], in0=outT[:, c, :, :],
                                in1=rstd[:, :], op=OP.mult)
        nc.vector.tensor_scalar(out=outT[:, c, :, :], in0=outT[:, c, :, :],
                                scalar1=gpack[:, c:c + 1],
                                scalar2=bpack[:, c:c + 1],
                                op0=OP.mult, op1=OP.add)


def _stage_b(st, s, pb, qb, spanT, ctxT, gnat_t, wm_t, gi_t, stageb_cb):
    """Batched (over b) fusion tail: o-proj, LN1, FFN, LN2, gates, merge."""
    nc, t = st.nc, st.t
    packs = st.packs
    hs_out = t["hs_out"]

    # ---- o = ctx @ wo + bo  (+ residual span) -> x1 ----
    x1 = pb.tile([P, HC, BPC, MAXL], BF16, tag="xT", bufs=2)
    for co in range(HC):
        wc = pb.tile([P, HC, P], BF16, tag="wcol", bufs=3)
        nc.sync.dma_start(
            out=wc[:], in_=t["w_wo"][:, co * P:(co + 1) * P]
            .rearrange("(c p) n -> p c n", p=P))
        po = qb.tile([P, NB], F32, tag="mmB", bufs=2)
        for ci in range(HC):
            nc.tensor.matmul(po[:, :], wc[:, ci, :], ctxT[:, ci, :, :],
                             start=(ci == 0), stop=False)
        nc.tensor.matmul(po[:, :], st.borow[:1, co * P:(co + 1) * P],
                         st.ones_row[:1, :], start=False, stop=True)
        nc.vector.tensor_tensor(out=x1[:, co, :, :], in0=po[:, :],
                                in1=spanT[:, co, :, :], op=OP.add)

    stageb_cb(0)

    # ---- LN1 ----
    o1 = pb.tile([P, HC, BPC, MAXL], BF16, tag="out1T", bufs=1)
    _layernorm_T(st, qb, pb, x1, o1, packs["p_g1"], packs["p_b1"])
    if DEBUG and s == 0:
        nc.sync.dma_start(out=t["d_x1"][:, :, :, :], in_=x1[:, :, :, :])
        nc.sync.dma_start(out=t["d_o1"][:, :, :, :], in_=o1[:, :, :, :])

    # ---- FFN: h = gelu(o1 @ fw1 + fb1); acc += h @ fw2 ----
    # NOTE: matmul start=True clears has_written for the whole PSUM bank,
    # so each accumulation group must own its bank exclusively for its
    # entire lifetime -> short consecutive groups + DVE adds into SBUF.
    GRP = 8
    acc = pb.tile([P, HC, NB], F32, tag="acc", bufs=1)
    for sup in range(FC // GRP):
        hfs, f2s = [], []
        for j in range(GRP):
            cf = sup * GRP + j
            f1 = pb.tile([P, HC, P], BF16, tag="f1c", bufs=3)
            nc.sync.dma_start(
                out=f1[:], in_=t["w_fw1"][:, cf * P:(cf + 1) * P]
                .rearrange("(c p) n -> p c n", p=P))
            ph = qb.tile([P, NB], F32, tag="mmB", bufs=2)
            for ci in range(HC):
                nc.tensor.matmul(ph[:, :], f1[:, ci, :], o1[:, ci, :, :],
                                 start=(ci == 0), stop=(ci == HC - 1))
            hf = pb.tile([P, NB], BF16, tag="hf", bufs=GRP + 1)
            nc.scalar.activation(hf[:, :], ph[:, :], AF.Gelu,
                                 bias=packs["p_fb1"][:, cf:cf + 1])
            f2c = pb.tile([P, H], BF16, tag="f2c", bufs=GRP + 1)
            nc.scalar.dma_start(out=f2c[:],
                                in_=t["w_fw2"][cf * P:(cf + 1) * P, :])
            hfs.append(hf)
            f2s.append(f2c)
        for co in range(HC):
            pacc = qb.tile([P, NB], F32, tag="acc2", bufs=2)
            for j in range(GRP):
                nc.tensor.matmul(pacc[:, :], f2s[j][:, co * P:(co + 1) * P],
                                 hfs[j][:, :], start=(j == 0),
                                 stop=(j == GRP - 1))
            if sup == 0:
                nc.vector.tensor_copy(acc[:, co, :], pacc[:, :])
            else:
                nc.vector.tensor_tensor(out=acc[:, co, :], in0=acc[:, co, :],
                                        in1=pacc[:, :], op=OP.add)

    # x2 = acc + fb2 + o1
    x2 = pb.tile([P, HC, BPC, MAXL], BF16, tag="xT", bufs=2)
    for co in range(HC):
        nc.vector.tensor_scalar(out=x2[:, co, :, :], in0=acc[:, co, :],
                                scalar1=packs["p_fb2"][:, co:co + 1],
                                scalar2=None, op0=OP.add)
        nc.vector.tensor_tensor(out=x2[:, co, :, :], in0=x2[:, co, :, :],
                                in1=o1[:, co, :, :], op=OP.add)

    if DEBUG and s == 0:
        nc.sync.dma_start(out=t["d_x2"][:, :, :, :], in_=x2[:, :, :, :])

    # ---- LN2 ----
    o2 = pb.tile([P, HC, BPC, MAXL], BF16, tag="out2T", bufs=1)
    _layernorm_T(st, qb, pb, x2, o2, packs["p_g2"], packs["p_b2"])

    # ---- gates ----
    gate = pb.tile([P, HC, BPC, MAXL], BF16, tag="gateT", bufs=1)
    for co in range(HC):
        wa = pb.tile([P, HC, P], BF16, tag="wcol", bufs=3)
        nc.sync.dma_start(
            out=wa[:], in_=t["w_gaw"][:, co * P:(co + 1) * P]
            .rearrange("(c p) n -> p c n", p=P))
        wt = pb.tile([P, HC, P], BF16, tag="wcol", bufs=3)
        nc.scalar.dma_start(
            out=wt[:], in_=t["w_gtw"][:, co * P:(co + 1) * P]
            .rearrange("(c p) n -> p c n", p=P))
        pg = qb.tile([P, NB], F32, tag="mmB", bufs=2)
        for ci in range(HC):
            nc.tensor.matmul(pg[:, :], wa[:, ci, :], o2[:, ci, :, :],
                             start=(ci == 0), stop=False)
        for ci in range(HC):
            nc.tensor.matmul(pg[:, :], wt[:, ci, :], spanT[:, ci, :, :],
                             start=False, stop=(ci == HC - 1))
        nc.scalar.activation(gate[:, co, :, :], pg[:, :], AF.Sigmoid,
                             bias=packs["p_gb"][:, co:co + 1])
    if DEBUG and s == 0:
        nc.sync.dma_start(out=t["d_o2"][:, :, :, :], in_=o2[:, :, :, :])
        nc.sync.dma_start(out=t["d_gate"][:, :, :, :], in_=gate[:, :, :, :])

    # ---- fused = span + gate*(o2 - span) ----
    fused = pb.tile([P, HC, BPC, MAXL], BF16, tag="fusedT", bufs=1)
    for co in range(HC):
        nc.vector.tensor_tensor(out=fused[:, co, :, :], in0=o2[:, co, :, :],
                                in1=spanT[:, co, :, :], op=OP.subtract)
        nc.vector.tensor_tensor(out=fused[:, co, :, :], in0=fused[:, co, :, :],
                                in1=gate[:, co, :, :], op=OP.mult)
        nc.vector.tensor_tensor(out=fused[:, co, :, :], in0=fused[:, co, :, :],
                                in1=spanT[:, co, :, :], op=OP.add)

    # ---- per-sample: back to natural, merge, scatter ----
    for b in range(BPC):
        fnat = pb.tile([MAXL, H], F32, tag="fnat", bufs=2)
        for c in range(0, HC, 2):
            pt = qb.tile([P, 2, P], BF16, tag="ptB", bufs=1)
            for j in range(2):
                nc.tensor.transpose(out=pt[0:MAXL, j, :],
                                    in_=fused[:, c + j, b, :],
                                    identity=st.identb[:, :])
            nc.scalar.copy(fnat[:, c * P:(c + 2) * P], pt[0:MAXL, :, :])
        merged = pb.tile([MAXL, H], F32, tag="merged", bufs=2)
        nc.vector.tensor_tensor(out=merged[:], in0=fnat[:], in1=gnat_t[b][:],
                                op=OP.subtract)
        nc.vector.tensor_scalar_mul(merged[:], merged[:], wm_t[b][:, :1])
        nc.vector.tensor_tensor(out=merged[:], in0=merged[:], in1=gnat_t[b][:],
                                op=OP.add)
        nc.gpsimd.indirect_dma_start(
            out=hs_out[:, :],
            out_offset=bass.IndirectOffsetOnAxis(ap=gi_t[b][:, :1], axis=0),
            in_=merged[:], in_offset=None)


# ============================ host glue ============================

_NC_CACHE = None


def _get_program():
    global _NC_CACHE
    if _NC_CACHE is None:
        _NC_CACHE = build_program()
    return _NC_CACHE


def _bf(x):
    return np.ascontiguousarray(np.asarray(x, np.float32).astype(NPBF))


def _fold_weights(inp):
    f64 = lambda x: np.asarray(x, np.float64)
    wk_eff = (f64(inp["mlp_w2"]) @ f64(inp["wk"])).astype(np.float32)
    wv_eff = (f64(inp["mlp_w2"]) @ f64(inp["wv"])).astype(np.float32)
    bv_eff = f64(inp["mlp_b2"]) @ f64(inp["wv"]) + f64(inp["bv"])
    bo_eff = (bv_eff @ f64(inp["wo"]) + f64(inp["bo"])).astype(np.float32)
    wq_s = (f64(inp["wq"]) * SCALE).astype(np.float32)
    bq_s = (f64(inp["bq"]) * SCALE).astype(np.float32)
    gb_eff = (f64(inp["ga_b"]) + f64(inp["gt_b"])).astype(np.float32)

    w = {}
    w["w_mw1"] = _bf(inp["mlp_w1"])
    w["w_wv"] = _bf(wv_eff)
    w["w_wq"] = _bf(wq_s)
    w["w_wo"] = _bf(inp["wo"])
    w["w_gaw"] = _bf(inp["ga_w"])
    w["w_gtw"] = _bf(inp["gt_w"])
    w["w_fw1"] = _bf(inp["ffn_w1"])
    w["w_fw2"] = _bf(inp["ffn_w2"])
    # wkT[p, hp, c, m] = wk_eff[c*128+m, hp*128+p]
    a = wk_eff.reshape(HC, P, HP, P)          # [c, m, hp, p]
    w["w_wkT"] = _bf(a.transpose(3, 2, 0, 1).reshape(P, HP * HC * P))

    def pack(vec, nch):
        return np.ascontiguousarray(
            np.asarray(vec, np.float32).reshape(nch, P).T)

    w["p_mb1"] = pack(inp["mlp_b1"], HC)
    w["p_bq"] = pack(bq_s, HC)
    w["p_fb1"] = pack(inp["ffn_b1"], FC)
    w["p_fb2"] = pack(inp["ffn_b2"], HC)
    w["p_gb"] = pack(gb_eff, HC)
    w["p_g1"] = pack(inp["ln1_g"], HC)
    w["p_b1"] = pack(inp["ln1_b"], HC)
    w["p_g2"] = pack(inp["ln2_g"], HC)
    w["p_b2"] = pack(inp["ln2_b"], HC)
    w["bo_row"] = _bf(bo_eff.reshape(1, H))
    w["ones_c"] = np.ones((P, 1), NPBF)
    w["ones_r"] = np.ones((1, NB), NPBF)
    return w


def _span_meta(spans, active, core):
    ar = np.arange(MAXL)
    gidx = np.zeros((NSPAN, BPC, MAXL), np.int32)
    vmsk = np.zeros((NSPAN, BPC, MAXL), np.float32)
    wmsk = np.zeros((NSPAN, BPC, MAXL), np.float32)
    for s in range(NSPAN):
        for bl in range(BPC):
            bg = core * BPC + bl
            stt = int(spans[bg, s, 0])
            en = min(int(spans[bg, s, 1]), S)
            L = max(en - stt, 0)
            idx = np.clip(stt + ar, 0, S - 1)
            gidx[s, bl] = bl * S + idx
            vmsk[s, bl] = (ar < L).astype(np.float32)
            wmsk[s, bl] = vmsk[s, bl] * np.float32(bool(active[bg, s]))
    return gidx, vmsk, wmsk


def _run(inputs, trace=False):
    nc = _get_program()
    hs = np.ascontiguousarray(inputs["hidden_states"], np.float32)
    au = np.asarray(inputs["audio_inputs"], np.float32).astype(NPBF)
    spans = np.asarray(inputs["spans_token_pos"])
    active = np.asarray(inputs["in_audios"])
    w = _fold_weights(inputs)

    in_maps = []
    for c in range(NCORES):
        gidx, vmsk, wmsk = _span_meta(spans, active, c)
        m = dict(w)
        m["hs_in"] = hs[c * BPC:(c + 1) * BPC].reshape(BPC * S, H)
        m["audio"] = np.ascontiguousarray(au[c * BPC:(c + 1) * BPC])
        m["gidx"], m["vmsk"], m["wmsk"] = gidx, vmsk, wmsk
        in_maps.append(m)

    kw = {}
    if trace:
        kw = dict(trace=True, trace_cores=[0])
    res = run_bass_kernel_spmd(nc, in_maps, core_ids=list(range(NCORES)), **kw)
    out = np.empty((B, S, H), np.float32)
    for c in range(NCORES):
        out[c * BPC:(c + 1) * BPC] = res.results[c]["hs_out"].reshape(BPC, S, H)
    return out, res


def kernel(**inputs):
    out, _ = _run(inputs, trace=False)
    return out


# revision 45
# speedup vs baseline: 1.5679x; 1.5222x over previous
"""Trainium2 Bass kernel for nn_AudioImaginationForGLUE (v2, bf16).

Pure data-parallel across 8 NeuronCores: each core handles 4 samples
(B=32 / 8).  Spans are two sequential phases (span 1 may read
hidden-state rows written by span 0).

Math transformations (vs reference):
  - audio-MLP second layer folded into K/V projections:
       wk_eff = mlp_w2 @ wk,  wv_eff = mlp_w2 @ wv
  - key bias dropped (softmax shift invariance along key axis)
  - value bias folded into output-proj bias (softmax rows sum to 1)
  - attention scale folded into wq, bq
  - NO max subtraction in softmax (scores are small; verified on host)
  - scoresT computed directly in [token, query] layout:
       scoresT = h1 @ WQ',   WQ'_h = wk_eff_h @ q_h^T
    so no K tensor, no attention transposes; the softmax normalization
    rides along the ctx matmul as a ones column appended to V.
  - ragged span handled by indirect-DMA gather/scatter with
    host-computed row indices.

Layout: activations transposed [feature -> partitions, tokens -> free].
Everything on the matmul path is bf16 (1 cycle/row on the PE, half the
DMA traffic, 2x DVE); accumulation is fp32 in PSUM.  Audio is loaded
with DMA-transpose (16-bit dtype) so the PE never transposes it.
"""

import numpy as np
import ml_dtypes

import concourse.bass as bass
import concourse.mybir as mybir
import concourse.tile as tile
from concourse import bacc
from concourse.masks import make_identity
from concourse.bass_utils import run_bass_kernel_spmd

F32 = mybir.dt.float32
BF16 = mybir.dt.bfloat16
F8 = mybir.dt.float8e4
PM_DR = mybir.MatmulPerfMode.DoubleRow
I32 = mybir.dt.int32
AF = mybir.ActivationFunctionType
AX = mybir.AxisListType
OP = mybir.AluOpType

P = 128
B, S, H, NH, FF, A, TA, NSPAN, MAXL = 32, 512, 768, 12, 3072, 768, 1024, 2, 64
DH = H // NH          # 64
HC = H // P           # 6 hidden chunks
HP = NH // 2          # 6 head pairs
FC = FF // P          # 24 ffn chunks
TT = TA // P          # 8 audio token tiles
NCORES = 8
BPC = B // NCORES     # 4 samples per core
NB = BPC * MAXL       # 256, stage-B token width
SCALE = 1.0 / float(np.sqrt(DH))
NPBF = ml_dtypes.bfloat16
NPF8 = ml_dtypes.float8_e4m3
# fp8 scale folding: mw1 x32, h1 carries x32; WQ' x128; wv_eff x64
S_MW1 = 32.0
S_WQ = 128.0
S_WV = 64.0


def build_program():
    nc = bacc.Bacc("TRN2", target_bir_lowering=False, debug=False)

    t = {}
    t["hs_in"] = nc.dram_tensor("hs_in", [BPC * S, H], F32, kind="ExternalInput")
    t["audio"] = nc.dram_tensor("audio", [BPC, NSPAN, TA, A], BF16,
                                kind="ExternalInput")
    for nm in ("w_wq", "w_wo", "w_gaw", "w_gtw"):
        t[nm] = nc.dram_tensor(nm, [H, H], BF16, kind="ExternalInput")
    for nm in ("w_mw1", "w_wv"):
        t[nm] = nc.dram_tensor(nm, [H, H], F8, kind="ExternalInput")
    t["w_wkT"] = nc.dram_tensor("w_wkT", [P, HP * HC * P], BF16,
                                kind="ExternalInput")
    t["w_fw1"] = nc.dram_tensor("w_fw1", [H, FF], BF16, kind="ExternalInput")
    t["w_fw2"] = nc.dram_tensor("w_fw2", [FF, H], BF16, kind="ExternalInput")
    for nm in ("p_mb1", "p_bq", "p_fb2", "p_gb", "p_g1", "p_b1", "p_g2", "p_b2"):
        t[nm] = nc.dram_tensor(nm, [P, HC], F32, kind="ExternalInput")
    t["p_fb1"] = nc.dram_tensor("p_fb1", [P, FC], F32, kind="ExternalInput")
    t["bo_row"] = nc.dram_tensor("bo_row", [1, H], BF16, kind="ExternalInput")
    t["ones_c"] = nc.dram_tensor("ones_c", [P, 1], BF16, kind="ExternalInput")
    t["ones_r"] = nc.dram_tensor("ones_r", [1, NB], BF16, kind="ExternalInput")
    t["gidx"] = nc.dram_tensor("gidx", [NSPAN, BPC, MAXL], I32, kind="ExternalInput")
    t["vmsk"] = nc.dram_tensor("vmsk", [NSPAN, BPC, MAXL], F32, kind="ExternalInput")
    t["wmsk"] = nc.dram_tensor("wmsk", [NSPAN, BPC, MAXL], F32, kind="ExternalInput")
    t["hs_out"] = nc.dram_tensor("hs_out", [BPC * S, H], F32, kind="ExternalOutput")
    if DEBUG:
        t["d_h1"] = nc.dram_tensor("d_h1", [P, HC, TA], BF16,
                                   kind="ExternalOutput")
        t["d_wqp"] = nc.dram_tensor("d_wqp", [P, HC, H], BF16,
                                    kind="ExternalOutput")
        t["d_esb"] = nc.dram_tensor("d_esb", [P, TT, H], BF16,
                                    kind="ExternalOutput")
        t["d_vaug"] = nc.dram_tensor("d_vaug", [P, TT, NH, DH + 1], BF16,
                                     kind="ExternalOutput")
        t["d_ctx"] = nc.dram_tensor("d_ctx", [MAXL, H], BF16,
                                    kind="ExternalOutput")
        t["d_span"] = nc.dram_tensor("d_span", [P, HC, BPC, MAXL], BF16,
                                     kind="ExternalOutput")
        t["d_qbd"] = nc.dram_tensor("d_qbd", [P, HP, BPC, 2, MAXL], BF16,
                                    kind="ExternalOutput")
        t["d_x1"] = nc.dram_tensor("d_x1", [P, HC, BPC, MAXL], BF16,
                                   kind="ExternalOutput")
        t["d_o1"] = nc.dram_tensor("d_o1", [P, HC, BPC, MAXL], BF16,
                                   kind="ExternalOutput")
        t["d_o2"] = nc.dram_tensor("d_o2", [P, HC, BPC, MAXL], BF16,
                                   kind="ExternalOutput")
        t["d_gate"] = nc.dram_tensor("d_gate", [P, HC, BPC, MAXL], BF16,
                                     kind="ExternalOutput")
        t["d_x2"] = nc.dram_tensor("d_x2", [P, HC, BPC, MAXL], BF16,
                                   kind="ExternalOutput")

    with tile.TileContext(nc) as tc, \
            nc.allow_low_precision("bf16 matmul path, fp32 accumulation"):
        _emit(nc, tc, t)
    nc.finalize()
    return nc


class _St:
    pass


def _emit(nc, tc, t):
    hs_in, hs_out = t["hs_in"], t["hs_out"]

    with (
        tc.tile_pool(name="const", bufs=1) as cpool,
        tc.tile_pool(name="resw", bufs=1) as resw,
        tc.tile_pool(name="perbs", bufs=1) as perbs,
    ):
        st = _St()
        st.nc, st.tc, st.t, st.perbs = nc, tc, t, perbs

        # ---- constants ----
        st.identb = cpool.tile([P, P], BF16, tag="identb")
        make_identity(nc, st.identb)
        st.ones_col = cpool.tile([P, 1], BF16, tag="ones_col")
        nc.sync.dma_start(out=st.ones_col[:], in_=t["ones_c"][:, :])
        st.ones_row = cpool.tile([1, NB], BF16, tag="ones_row")
        nc.sync.dma_start(out=st.ones_row[:], in_=t["ones_r"][:, :])
        st.eps_t = cpool.tile([P, 1], F32, tag="eps_t")
        nc.vector.memset(st.eps_t[:], 1e-5)

        st.packs = {}
        for nm in ("p_mb1", "p_bq", "p_fb1", "p_fb2", "p_gb",
                   "p_g1", "p_b1", "p_g2", "p_b2"):
            nch = FC if nm == "p_fb1" else HC
            pk = cpool.tile([P, nch], F32, tag=nm)
            nc.sync.dma_start(out=pk[:], in_=t[nm][:, :])
            st.packs[nm] = pk
        st.borow = cpool.tile([1, H], BF16, tag="borow")
        nc.sync.dma_start(out=st.borow[:], in_=t["bo_row"][:, :])

        # ---- resident weights ----
        st.wres = {}
        for nm, dram, dt_ in (("mw1", t["w_mw1"], F8), ("wv", t["w_wv"], F8),
                              ("wq", t["w_wq"], BF16)):
            ws = resw.tile([P, HC, H], dt_, tag="w_" + nm)
            nc.sync.dma_start(
                out=ws[:], in_=dram[:, :].rearrange("(c p) n -> p c n", p=P))
            st.wres[nm] = ws
        wkT = resw.tile([P, HP, HC, P], BF16, tag="w_wkT")
        nc.sync.dma_start(
            out=wkT[:], in_=t["w_wkT"][:, :].rearrange(
                "p (h c n) -> p h c n", h=HP, c=HC))
        st.wres["wkT"] = wkT

        # ---- full hidden-state copy in -> out (8 chunks) ----
        rows = BPC * S
        step = rows // 8
        for i in range(8):
            nc.sync.dma_start(out=hs_out[i * step:(i + 1) * step, :],
                              in_=hs_in[i * step:(i + 1) * step, :])

        # ---- prologue: audio + MLP1 for (span 0, sample 0) ----
        with tc.tile_pool(name="pstg", bufs=1, space="PSUM") as pstg:
            st.aiT_next = _audio_load(st, 0, 0, pstg)
            st.h1_next = _mlp1(st, st.aiT_next, pstg)

        for s in range(NSPAN):
            _emit_span(st, s)


USE_DMA_T = True
DEBUG = False


def _audio_load(st, s, b, qpool):
    """Load one sample-span of audio into [feat, tok] layout."""
    nc = st.nc
    aiT = st.perbs.tile([P, HC, TA], BF16, tag="aiT", bufs=2)
    if USE_DMA_T:
        for c in range(HC):
            nc.sync.dma_start(out=aiT[:, c, :],
                              in_=st.t["audio"][b, s, :, c * P:(c + 1) * P],
                              transpose=True)
        return aiT
    for tt in range(TT):
        anat = st.perbs.tile([P, A], BF16, tag="anat", bufs=3)
        nc.sync.dma_start(out=anat[:], in_=st.t["audio"][b, s,
                                                         tt * P:(tt + 1) * P, :])
        for c in range(0, HC, 2):
            pt = qpool.tile([P, 2, P], BF16, tag="tp", bufs=2)
            for j in range(2):
                nc.tensor.transpose(out=pt[:, j, :],
                                    in_=anat[:, (c + j) * P:(c + j + 1) * P],
                                    identity=st.identb[:, :])
            nc.scalar.copy(aiT[:, c:c + 2, tt * P:(tt + 1) * P],
                           pt[:, :, :])
    return aiT


def _mlp1(st, aiT, qpool):
    """h1T = relu(aiT.T @ mw1_x32 + mb1*32), fp8, carrying a 32x scale.

    aiT arrives bf16 from the DMA-transpose; quantize to fp8 first
    (DVE copy), then 3 DoubleRow matmuls per output (K=256 each).
    """
    nc = st.nc
    ai8 = st.perbs.tile([P, HC, TA], F8, tag="ai8", bufs=2)
    nc.vector.tensor_copy(ai8[:, :, :], aiT[:, :, :])
    h1T = st.perbs.tile([P, HC, TA], F8, tag="h1T", bufs=2)
    for co in range(HC):
        for half in range(2):
            ph = qpool.tile([P, 1024], F32, tag="mm", bufs=2)
            for c2 in range(HC // 2):
                nc.tensor.matmul(
                    ph[:, 0:512],
                    st.wres["mw1"][:, 2 * c2:2 * c2 + 2, co * P:(co + 1) * P],
                    ai8[:, 2 * c2:2 * c2 + 2, half * 512:(half + 1) * 512],
                    start=(c2 == 0), stop=(c2 == HC // 2 - 1),
                    perf_mode=PM_DR)
            nc.vector.tensor_scalar(
                out=h1T[:, co, half * 512:(half + 1) * 512], in0=ph[:, 0:512],
                scalar1=st.packs["p_mb1"][:, co:co + 1], scalar2=0.0,
                op0=OP.add, op1=OP.max)
    return h1T


def _emit_span(st, s):
    nc, perbs = st.nc, st.perbs

    spanT = perbs.tile([P, HC, BPC, MAXL], BF16, tag="spanT")
    ctxT = perbs.tile([P, HC, BPC, MAXL], BF16, tag="ctxT")
    gnat_t = [None] * BPC
    wm_t = [None] * BPC
    gi_t = [None] * BPC

    with (
        st.tc.tile_pool(name=f"sA{s}", bufs=1) as pa,
        st.tc.tile_pool(name=f"psA{s}", bufs=1, space="PSUM") as qa,
    ):
        qh = _phase_head(st, s, pa, qa, spanT, gnat_t, wm_t, gi_t)
        for b in range(BPC):
            if st.h1_next is not None:
                aiT, h1T = st.aiT_next, st.h1_next
                st.aiT_next = st.h1_next = None
            else:
                aiT = st.aiT_next if st.aiT_next is not None else \
                    _audio_load(st, s, b, qa)
                st.aiT_next = None
                h1T = _mlp1(st, aiT, qa)
            if b + 1 < BPC:
                st.aiT_next = _audio_load(st, s, b + 1, qa)
            _attend(st, s, b, pa, qa, qh, h1T, spanT, ctxT)

    with (
        st.tc.tile_pool(name=f"sB{s}", bufs=1) as pb,
        st.tc.tile_pool(name=f"psB{s}", bufs=1, space="PSUM") as qb,
    ):
        def stageb_cb(point, s=s):
            # prefetch next span's first audio block (DMA-transpose only,
            # no PSUM needed)
            if s + 1 < NSPAN and point == 0:
                st.aiT_next = _audio_load(st, s + 1, 0, qb)

        _stage_b(st, s, pb, qb, spanT, ctxT, gnat_t, wm_t, gi_t, stageb_cb)


def _phase_head(st, s, pa, qa, spanT, gnat_t, wm_t, gi_t):
    """Gather the 4 span windows, build spanT, batched q projection."""
    nc, t = st.nc, st.t
    perbs = st.perbs
    src = t["hs_in"] if s == 0 else t["hs_out"]
    for b in range(BPC):
        gi = perbs.tile([MAXL, 1], I32, tag="gi", bufs=4)
        nc.sync.dma_start(out=gi[:],
                          in_=t["gidx"][s, b, :].rearrange("(p o) -> p o", o=1))
        vm = perbs.tile([MAXL, 1], F32, tag="vm", bufs=4)
        nc.sync.dma_start(out=vm[:],
                          in_=t["vmsk"][s, b, :].rearrange("(p o) -> p o", o=1))
        wm = perbs.tile([MAXL, 1], F32, tag="wm", bufs=4)
        nc.sync.dma_start(out=wm[:],
                          in_=t["wmsk"][s, b, :].rearrange("(p o) -> p o", o=1))
        gnat = perbs.tile([MAXL, H], F32, tag="gnat", bufs=4)
        nc.gpsimd.indirect_dma_start(
            out=gnat[:], out_offset=None, in_=src[:, :],
            in_offset=bass.IndirectOffsetOnAxis(ap=gi[:, :1], axis=0))
        gnat_t[b], wm_t[b], gi_t[b] = gnat, wm, gi

        snat = pa.tile([MAXL, H], BF16, tag="snat", bufs=2)
        nc.vector.tensor_scalar_mul(snat[:], gnat[:], vm[:, :1])
        for c in range(0, HC, 2):
            pt = qa.tile([P, 2, P], BF16, tag="tp", bufs=2)
            for j in range(2):
                nc.tensor.transpose(out=pt[:, j, 0:MAXL],
                                    in_=snat[:, (c + j) * P:(c + j + 1) * P],
                                    identity=st.identb[:MAXL, :MAXL])
            nc.scalar.copy(spanT[:, c:c + 2, b, :], pt[:, :, 0:MAXL])

    # batched q projection into block-diagonal layout: for each head pair
    # the [128, 2, MAXL] slice [:, hp, b, :, :] is [[qA, 0], [0, qB]], so a
    # single K=128 matmul against the wkT pair computes both heads' WQ'
    # columns without mixing them.
    qbd = pa.tile([P, HP, BPC, 2, MAXL], BF16, tag="qbd", bufs=1)
    nc.vector.memset(qbd[0:DH, :, :, 1, :], 0.0)
    nc.vector.memset(qbd[DH:P, :, :, 0, :], 0.0)
    for co in range(HC):
        pq = qa.tile([P, 1024], F32, tag="mm", bufs=2)
        for ci in range(HC):
            nc.tensor.matmul(pq[:, 0:NB],
                             st.wres["wq"][:, ci, co * P:(co + 1) * P],
                             spanT[:, ci, :, :],
                             start=(ci == 0), stop=(ci == HC - 1))
        nc.scalar.activation(qbd[0:DH, co, :, 0, :], pq[0:DH, 0:NB],
                             AF.Identity, bias=st.packs["p_bq"][0:DH, co:co + 1])
        nc.scalar.activation(qbd[DH:P, co, :, 1, :], pq[DH:P, 0:NB],
                             AF.Identity, bias=st.packs["p_bq"][DH:P, co:co + 1])
    if DEBUG and s == 0:
        nc.sync.dma_start(out=t["d_span"][:, :, :, :], in_=spanT[:, :, :, :])
        nc.sync.dma_start(out=t["d_qbd"][:, :, :, :, :],
                          in_=qbd[:, :, :, :, :])
    return qbd


def _attend(st, s, b, pa, qa, qh, h1T, spanT, ctxT):
    """WQ' -> scoresT -> exp -> V -> ctx for one sample."""
    nc = st.nc
    wkT = st.wres["wkT"]

    # ---- WQ'[feat(c chunk), (head, q)] = wk_eff_h @ q_h^T, fp8 x128 ----
    wqp = pa.tile([P, HC, H], F8, tag="wqp", bufs=1)
    for c in range(HC):
        pw = qa.tile([P, 1024], F32, tag="mm", bufs=2)
        for hp in range(HP):
            nc.tensor.matmul(pw[:, hp * P:(hp + 1) * P],
                             wkT[:, hp, c, :], qh[:, hp, b, :, :],
                             start=True, stop=True)
        nc.scalar.mul(wqp[:, c, :], pw[:, 0:H], S_WQ)

    # ---- scoresT tiles + exp -> esbT [tok, (head, q)], bf16 ----
    # scoresT carries x(32*128); fold out in the exp's input scale.
    esbT = pa.tile([P, TT, H], BF16, tag="esbT", bufs=1)
    for tt in range(TT):
        ps = qa.tile([P, 1024], F32, tag="mm", bufs=2)
        for c2 in range(HC // 2):
            lhs = h1T[:, 2 * c2:2 * c2 + 2, tt * P:(tt + 1) * P]
            nc.tensor.matmul(ps[:, 0:512], lhs,
                             wqp[:, 2 * c2:2 * c2 + 2, 0:512],
                             start=(c2 == 0), stop=(c2 == HC // 2 - 1),
                             perf_mode=PM_DR)
            nc.tensor.matmul(ps[:, 512:H], lhs,
                             wqp[:, 2 * c2:2 * c2 + 2, 512:H],
                             start=(c2 == 0), stop=(c2 == HC // 2 - 1),
                             perf_mode=PM_DR)
        nc.scalar.activation(esbT[:, tt, :], ps[:, 0:H], AF.Exp,
                             scale=1.0 / (S_MW1 * S_WQ))

    # ---- V with ones column -> v_aug [tok, tt, head, 65], bf16 ----
    # V carries x(32*64); fold out in the copy.
    v_aug = pa.tile([P, TT, NH, DH + 1], BF16, tag="v_aug", bufs=1)
    nc.vector.memset(v_aug[:, :, :, DH:DH + 1], 1.0)
    for tt in range(TT):
        pv = qa.tile([P, 1024], F32, tag="mm", bufs=2)
        for c2 in range(HC // 2):
            lhs = h1T[:, 2 * c2:2 * c2 + 2, tt * P:(tt + 1) * P]
            nc.tensor.matmul(pv[:, 0:512], lhs,
                             st.wres["wv"][:, 2 * c2:2 * c2 + 2, 0:512],
                             start=(c2 == 0), stop=(c2 == HC // 2 - 1),
                             perf_mode=PM_DR)
            nc.tensor.matmul(pv[:, 512:H], lhs,
                             st.wres["wv"][:, 2 * c2:2 * c2 + 2, 512:H],
                             start=(c2 == 0), stop=(c2 == HC // 2 - 1),
                             perf_mode=PM_DR)
        nc.vector.tensor_scalar(
            out=v_aug[:, tt, 0:8, 0:DH],
            in0=pv[:, 0:512].rearrange("p (h d) -> p h d", h=8),
            scalar1=1.0 / (S_MW1 * S_WV), scalar2=None, op0=OP.mult)
        nc.vector.tensor_scalar(
            out=v_aug[:, tt, 8:NH, 0:DH],
            in0=pv[:, 512:H].rearrange("p (h d) -> p h d", h=4),
            scalar1=1.0 / (S_MW1 * S_WV), scalar2=None, op0=OP.mult)

    # ---- ctx + normalization -> ctxT[:, :, b, :] ----
    ctx_nat = pa.tile([MAXL, H], BF16, tag="ctx_nat", bufs=2)
    for hh in range(2):
        pc = qa.tile([MAXL, 512], F32, tag="ctxp", bufs=2)
        for h6 in range(6):
            h = hh * 6 + h6
            for tt in range(TT):
                nc.tensor.matmul(pc[:, h6 * (DH + 1):(h6 + 1) * (DH + 1)],
                                 esbT[:, tt, h * DH:(h + 1) * DH],
                                 v_aug[:, tt, h, :],
                                 start=(tt == 0), stop=(tt == TT - 1))
        for h6 in range(6):
            h = hh * 6 + h6
            rec = pa.tile([MAXL, 1], F32, tag="rec", bufs=4)
            nc.vector.reciprocal(
                rec[:], pc[:, h6 * (DH + 1) + DH:h6 * (DH + 1) + DH + 1])
            nc.vector.tensor_scalar_mul(
                ctx_nat[:, h * DH:(h + 1) * DH],
                pc[:, h6 * (DH + 1):h6 * (DH + 1) + DH], rec[:, :1])
    for c in range(0, HC, 2):
        pt = qa.tile([P, 2, P], BF16, tag="tp", bufs=2)
        for j in range(2):
            nc.tensor.transpose(out=pt[:, j, 0:MAXL],
                                in_=ctx_nat[:, (c + j) * P:(c + j + 1) * P],
                                identity=st.identb[:MAXL, :MAXL])
        nc.scalar.copy(ctxT[:, c:c + 2, b, :], pt[:, :, 0:MAXL])

    if DEBUG and s == 0 and b == 0:
        nc.sync.dma_start(out=st.t["d_h1"][:, :, :], in_=h1T[:, :, :])
        nc.sync.dma_start(out=st.t["d_wqp"][:, :, :], in_=wqp[:, :, :])
        nc.sync.dma_start(out=st.t["d_esb"][:, :, :], in_=esbT[:, :, :])
        nc.sync.dma_start(out=st.t["d_vaug"][:, :, :, :], in_=v_aug[:, :, :, :])
        nc.sync.dma_start(out=st.t["d_ctx"][:, :], in_=ctx_nat[:, :])


def _layernorm_T(st, qb, pb, xT, outT, gpack, bpack):
    """LayerNorm over the partition (feature) axis of xT [128, HC, BPC, MAXL].

    xT is bf16; column stats via ones-matmul, partition-broadcast of the
    normalization rows via rank-1 matmuls.
    """
    nc = st.nc
    pst = qb.tile([1, 2, NB], F32, tag="st", bufs=1)
    for c in range(HC):
        nc.tensor.matmul(pst[:, 0, :], st.ones_col[:, :], xT[:, c, :, :],
                         start=(c == 0), stop=(c == HC - 1))
    m_row = pb.tile([1, NB], BF16, tag="m_row", bufs=1)
    nc.vector.tensor_scalar_mul(m_row[:], pst[:, 0, :], 1.0 / H)

    sq = pb.tile([P, HC, NB], BF16, tag="sq", bufs=1)
    for c in range(HC):
        nc.scalar.activation(sq[:, c, :], xT[:, c, :, :], AF.Square)
    for c in range(HC):
        nc.tensor.matmul(pst[:, 1, :], st.ones_col[:, :], sq[:, c, :],
                         start=(c == 0), stop=(c == HC - 1))
    var = pb.tile([1, NB], BF16, tag="var", bufs=1)
    msq = pb.tile([1, NB], F32, tag="msq", bufs=1)
    nc.scalar.activation(msq[:], m_row[:], AF.Square)
    nc.vector.tensor_scalar(out=var[:], in0=pst[:, 1, :], scalar1=1.0 / H,
                            scalar2=None, op0=OP.mult)
    nc.vector.tensor_tensor(out=var[:], in0=var[:], in1=msq[:],
                            op=OP.subtract)
    # broadcast mean and variance to all partitions via rank-1 matmuls
    pmv = qb.tile([P, 2, NB], F32, tag="st2", bufs=1)
    nc.tensor.matmul(pmv[:, 0, :], st.ones_row[:1, :P], m_row[:1, :],
                     start=True, stop=True)
    nc.tensor.matmul(pmv[:, 1, :], st.ones_row[:1, :P], var[:1, :],
                     start=True, stop=True)
    rstd = pb.tile([P, NB], F32, tag="rstd", bufs=1)
    nc.scalar.activation(rstd[:], pmv[:, 1, :], AF.Sqrt, bias=st.eps_t[:, :1])
    nc.vector.reciprocal(rstd[:], rstd[:])
    for c in range(HC):
        nc.vector.tensor_tensor(out=outT[:, c, :, :], in0=xT[:, c, :, :],
                                in1=pmv[:, 0, :], op=OP.subtract)
        nc.vector.tensor_tensor(out=outT[:, c, :, :], in0=outT[:, c, :, :],
                                in1=rstd[:, :], op=OP.mult)
        nc.vector.tensor_scalar(out=outT[:, c, :, :], in0=outT[:, c, :, :],
                                scalar1=gpack[:, c:c + 1],
                                scalar2=bpack[:, c:c + 1],
                                op0=OP.mult, op1=OP.add)


def _stage_b(st, s, pb, qb, spanT, ctxT, gnat_t, wm_t, gi_t, stageb_cb):
    """Batched (over b) fusion tail: o-proj, LN1, FFN, LN2, gates, merge."""
    nc, t = st.nc, st.t
    packs = st.packs
    hs_out = t["hs_out"]

    # ---- o = ctx @ wo + bo  (+ residual span) -> x1 ----
    x1 = pb.tile([P, HC, BPC, MAXL], BF16, tag="xT", bufs=2)
    for co in range(HC):
        wc = pb.tile([P, HC, P], BF16, tag="wcol", bufs=3)
        nc.sync.dma_start(
            out=wc[:], in_=t["w_wo"][:, co * P:(co + 1) * P]
            .rearrange("(c p) n -> p c n", p=P))
        po = qb.tile([P, NB], F32, tag="mmB", bufs=2)
        for ci in range(HC):
            nc.tensor.matmul(po[:, :], wc[:, ci, :], ctxT[:, ci, :, :],
                             start=(ci == 0), stop=False)
        nc.tensor.matmul(po[:, :], st.borow[:1, co * P:(co + 1) * P],
                         st.ones_row[:1, :], start=False, stop=True)
        nc.vector.tensor_tensor(out=x1[:, co, :, :], in0=po[:, :],
                                in1=spanT[:, co, :, :], op=OP.add)

    stageb_cb(0)

    # ---- LN1 ----
    o1 = pb.tile([P, HC, BPC, MAXL], BF16, tag="out1T", bufs=1)
    _layernorm_T(st, qb, pb, x1, o1, packs["p_g1"], packs["p_b1"])
    if DEBUG and s == 0:
        nc.sync.dma_start(out=t["d_x1"][:, :, :, :], in_=x1[:, :, :, :])
        nc.sync.dma_start(out=t["d_o1"][:, :, :, :], in_=o1[:, :, :, :])

    # ---- FFN: h = gelu(o1 @ fw1 + fb1); acc += h @ fw2 ----
    # NOTE: matmul start=True clears has_written for the whole PSUM bank,
    # so each accumulation group must own its bank exclusively for its
    # entire lifetime -> short consecutive groups + DVE adds into SBUF.
    GRP = 8
    acc = pb.tile([P, HC, NB], F32, tag="acc", bufs=1)
    for sup in range(FC // GRP):
        hfs, f2s = [], []
        for j in range(GRP):
            cf = sup * GRP + j
            f1 = pb.tile([P, HC, P], BF16, tag="f1c", bufs=3)
            nc.sync.dma_start(
                out=f1[:], in_=t["w_fw1"][:, cf * P:(cf + 1) * P]
                .rearrange("(c p) n -> p c n", p=P))
            ph = qb.tile([P, NB], F32, tag="mmB", bufs=2)
            for ci in range(HC):
                nc.tensor.matmul(ph[:, :], f1[:, ci, :], o1[:, ci, :, :],
                                 start=(ci == 0), stop=(ci == HC - 1))
            hf = pb.tile([P, NB], BF16, tag="hf", bufs=GRP + 1)
            nc.scalar.activation(hf[:, :], ph[:, :], AF.Gelu,
                                 bias=packs["p_fb1"][:, cf:cf + 1])
            f2c = pb.tile([P, H], BF16, tag="f2c", bufs=GRP + 1)
            nc.scalar.dma_start(out=f2c[:],
                                in_=t["w_fw2"][cf * P:(cf + 1) * P, :])
            hfs.append(hf)
            f2s.append(f2c)
        for co in range(HC):
            pacc = qb.tile([P, NB], F32, tag="acc2", bufs=2)
            for j in range(GRP):
                nc.tensor.matmul(pacc[:, :], f2s[j][:, co * P:(co + 1) * P],
                                 hfs[j][:, :], start=(j == 0),
                                 stop=(j == GRP - 1))
            if sup == 0:
                nc.vector.tensor_copy(acc[:, co, :], pacc[:, :])
            else:
                nc.vector.tensor_tensor(out=acc[:, co, :], in0=acc[:, co, :],
                                        in1=pacc[:, :], op=OP.add)

    # x2 = acc + fb2 + o1
    x2 = pb.tile([P, HC, BPC, MAXL], BF16, tag="xT", bufs=2)
    for co in range(HC):
        nc.vector.tensor_scalar(out=x2[:, co, :, :], in0=acc[:, co, :],
                                scalar1=packs["p_fb2"][:, co:co + 1],
                                scalar2=None, op0=OP.add)
        nc.vector.tensor_tensor(out=x2[:, co, :, :], in0=x2[:, co, :, :],
                                in1=o1[:, co, :, :], op=OP.add)

    if DEBUG and s == 0:
        nc.sync.dma_start(out=t["d_x2"][:, :, :, :], in_=x2[:, :, :, :])

    # ---- LN2 ----
    o2 = pb.tile([P, HC, BPC, MAXL], BF16, tag="out2T", bufs=1)
    _layernorm_T(st, qb, pb, x2, o2, packs["p_g2"], packs["p_b2"])

    # ---- gates ----
    gate = pb.tile([P, HC, BPC, MAXL], BF16, tag="gateT", bufs=1)
    for co in range(HC):
        wa = pb.tile([P, HC, P], BF16, tag="wcol", bufs=3)
        nc.sync.dma_start(
            out=wa[:], in_=t["w_gaw"][:, co * P:(co + 1) * P]
            .rearrange("(c p) n -> p c n", p=P))
        wt = pb.tile([P, HC, P], BF16, tag="wcol", bufs=3)
        nc.scalar.dma_start(
            out=wt[:], in_=t["w_gtw"][:, co * P:(co + 1) * P]
            .rearrange("(c p) n -> p c n", p=P))
        pg = qb.tile([P, NB], F32, tag="mmB", bufs=2)
        for ci in range(HC):
            nc.tensor.matmul(pg[:, :], wa[:, ci, :], o2[:, ci, :, :],
                             start=(ci == 0), stop=False)
        for ci in range(HC):
            nc.tensor.matmul(pg[:, :], wt[:, ci, :], spanT[:, ci, :, :],
                             start=False, stop=(ci == HC - 1))
        nc.scalar.activation(gate[:, co, :, :], pg[:, :], AF.Sigmoid,
                             bias=packs["p_gb"][:, co:co + 1])
    if DEBUG and s == 0:
        nc.sync.dma_start(out=t["d_o2"][:, :, :, :], in_=o2[:, :, :, :])
        nc.sync.dma_start(out=t["d_gate"][:, :, :, :], in_=gate[:, :, :, :])

    # ---- fused = span + gate*(o2 - span) ----
    fused = pb.tile([P, HC, BPC, MAXL], BF16, tag="fusedT", bufs=1)
    for co in range(HC):
        nc.vector.tensor_tensor(out=fused[:, co, :, :], in0=o2[:, co, :, :],
                                in1=spanT[:, co, :, :], op=OP.subtract)
        nc.vector.tensor_tensor(out=fused[:, co, :, :], in0=fused[:, co, :, :],
                                in1=gate[:, co, :, :], op=OP.mult)
        nc.vector.tensor_tensor(out=fused[:, co, :, :], in0=fused[:, co, :, :],
                                in1=spanT[:, co, :, :], op=OP.add)

    # ---- per-sample: back to natural, merge, scatter ----
    for b in range(BPC):
        fnat = pb.tile([MAXL, H], F32, tag="fnat", bufs=2)
        for c in range(0, HC, 2):
            pt = qb.tile([P, 2, P], BF16, tag="ptB", bufs=1)
            for j in range(2):
                nc.tensor.transpose(out=pt[0:MAXL, j, :],
                                    in_=fused[:, c + j, b, :],
                                    identity=st.identb[:, :])
            nc.scalar.copy(fnat[:, c * P:(c + 2) * P], pt[0:MAXL, :, :])
        merged = pb.tile([MAXL, H], F32, tag="merged", bufs=2)
        nc.vector.tensor_tensor(out=merged[:], in0=fnat[:], in1=gnat_t[b][:],
                                op=OP.subtract)
        nc.vector.tensor_scalar_mul(merged[:], merged[:], wm_t[b][:, :1])
        nc.vector.tensor_tensor(out=merged[:], in0=merged[:], in1=gnat_t[b][:],
                                op=OP.add)
        nc.gpsimd.indirect_dma_start(
            out=hs_out[:, :],
            out_offset=bass.IndirectOffsetOnAxis(ap=gi_t[b][:, :1], axis=0),
            in_=merged[:], in_offset=None)


# ============================ host glue ============================

_NC_CACHE = None


def _get_program():
    global _NC_CACHE
    if _NC_CACHE is None:
        _NC_CACHE = build_program()
    return _NC_CACHE


def _bf(x):
    return np.ascontiguousarray(np.asarray(x, np.float32).astype(NPBF))


def _fold_weights(inp):
    f64 = lambda x: np.asarray(x, np.float64)
    wk_eff = (f64(inp["mlp_w2"]) @ f64(inp["wk"])).astype(np.float32)
    wv_eff = (f64(inp["mlp_w2"]) @ f64(inp["wv"])).astype(np.float32)
    bv_eff = f64(inp["mlp_b2"]) @ f64(inp["wv"]) + f64(inp["bv"])
    bo_eff = (bv_eff @ f64(inp["wo"]) + f64(inp["bo"])).astype(np.float32)
    wq_s = (f64(inp["wq"]) * SCALE).astype(np.float32)
    bq_s = (f64(inp["bq"]) * SCALE).astype(np.float32)
    gb_eff = (f64(inp["ga_b"]) + f64(inp["gt_b"])).astype(np.float32)

    def _f8(x, s):
        return np.ascontiguousarray(
            np.clip(np.asarray(x, np.float32) * s, -240.0, 240.0).astype(NPF8))

    w = {}
    w["w_mw1"] = _f8(inp["mlp_w1"], S_MW1)
    w["w_wv"] = _f8(wv_eff, S_WV)
    w["w_wq"] = _bf(wq_s)
    w["w_wo"] = _bf(inp["wo"])
    w["w_gaw"] = _bf(inp["ga_w"])
    w["w_gtw"] = _bf(inp["gt_w"])
    w["w_fw1"] = _bf(inp["ffn_w1"])
    w["w_fw2"] = _bf(inp["ffn_w2"])
    # wkT[p, hp, c, m] = wk_eff[c*128+m, hp*128+p]
    a = wk_eff.reshape(HC, P, HP, P)          # [c, m, hp, p]
    w["w_wkT"] = _bf(a.transpose(3, 2, 0, 1).reshape(P, HP * HC * P))

    def pack(vec, nch):
        return np.ascontiguousarray(
            np.asarray(vec, np.float32).reshape(nch, P).T)

    w["p_mb1"] = pack(np.asarray(inp["mlp_b1"], np.float32) * S_MW1, HC)
    w["p_bq"] = pack(bq_s, HC)
    w["p_fb1"] = pack(inp["ffn_b1"], FC)
    w["p_fb2"] = pack(inp["ffn_b2"], HC)
    w["p_gb"] = pack(gb_eff, HC)
    w["p_g1"] = pack(inp["ln1_g"], HC)
    w["p_b1"] = pack(inp["ln1_b"], HC)
    w["p_g2"] = pack(inp["ln2_g"], HC)
    w["p_b2"] = pack(inp["ln2_b"], HC)
    w["bo_row"] = _bf(bo_eff.reshape(1, H))
    w["ones_c"] = np.ones((P, 1), NPBF)
    w["ones_r"] = np.ones((1, NB), NPBF)
    return w


def _span_meta(spans, active, core):
    ar = np.arange(MAXL)
    gidx = np.zeros((NSPAN, BPC, MAXL), np.int32)
    vmsk = np.zeros((NSPAN, BPC, MAXL), np.float32)
    wmsk = np.zeros((NSPAN, BPC, MAXL), np.float32)
    for s in range(NSPAN):
        for bl in range(BPC):
            bg = core * BPC + bl
            stt = int(spans[bg, s, 0])
            en = min(int(spans[bg, s, 1]), S)
            L = max(en - stt, 0)
            idx = np.clip(stt + ar, 0, S - 1)
            gidx[s, bl] = bl * S + idx
            vmsk[s, bl] = (ar < L).astype(np.float32)
            wmsk[s, bl] = vmsk[s, bl] * np.float32(bool(active[bg, s]))
    return gidx, vmsk, wmsk


def _run(inputs, trace=False):
    nc = _get_program()
    hs = np.ascontiguousarray(inputs["hidden_states"], np.float32)
    au = np.asarray(inputs["audio_inputs"], np.float32).astype(NPBF)
    spans = np.asarray(inputs["spans_token_pos"])
    active = np.asarray(inputs["in_audios"])
    w = _fold_weights(inputs)

    in_maps = []
    for c in range(NCORES):
        gidx, vmsk, wmsk = _span_meta(spans, active, c)
        m = dict(w)
        m["hs_in"] = hs[c * BPC:(c + 1) * BPC].reshape(BPC * S, H)
        m["audio"] = np.ascontiguousarray(au[c * BPC:(c + 1) * BPC])
        m["gidx"], m["vmsk"], m["wmsk"] = gidx, vmsk, wmsk
        in_maps.append(m)

    kw = {}
    if trace:
        kw = dict(trace=True, trace_cores=[0])
    res = run_bass_kernel_spmd(nc, in_maps, core_ids=list(range(NCORES)), **kw)
    out = np.empty((B, S, H), np.float32)
    for c in range(NCORES):
        out[c * BPC:(c + 1) * BPC] = res.results[c]["hs_out"].reshape(BPC, S, H)
    return out, res


def kernel(**inputs):
    out, _ = _run(inputs, trace=False)
    return out
